# revision 8
# baseline (speedup 1.0000x reference)
"""Trainium2 Bass kernel for nn_Graph_Layer_44787918963014 (gnn_message_passing).

out = ALPHA * softmax(q k^T) @ x @ weight + (1-ALPHA) * G_time @ x @ weight_time
with q = x@W0.T, k = x@W1.T, G_time the normalized (n-|i-j|) Toeplitz affinity.

Strategy (8 NeuronCores, row-sharded: core c owns queries [c*1024, (c+1)*1024)):
  host prep : q/k projections (cheap [N,512]@[512,128] GEMMs), bf16 hi+lo split
              (fp32-accurate scores from 3 bf16 matmuls); global constant score
              shift c (softmax-invariant, estimated from sampled rows, huge fp32
              margin); G_time @ x computed EXACTLY in O(N*D) via prefix sums
              (Toeplitz structure), so the time branch needs no N x N work.
  device    : per j-block of 128 keys -> scores S^T[j,m] in fp32 PSUM (3 bf16
              matmuls); exp(S^T - c) on ACT -> bf16 E^T; Z partials (DVE);
              U^T[d,m] += x_j^T E_j accumulated across all 64 j-blocks directly
              in PSUM (no SBUF flushes); Z partition-reduce via ones-matmul,
              reciprocal (DVE), partition-broadcast (GPSIMD); U^T scaled by 1/Z;
              single fused projection outT = [a*W; (1-a)*Wt]^T @ [U^T/Z; trT].
  host epi  : out[rows] = outT.T  (transpose only).

Self-contained: shapes hardcoded, no sibling imports. Falls back to an exact
blocked host computation if the device path fails for any reason.
"""
import sys, time, traceback
import numpy as np

N, IN, FEAT, NOUT = 8192, 512, 128, 512
ALPHA = 0.5
NCORES = 8
NLOC = N // NCORES     # 1024 queries per core
P = 128
NBLK = N // P          # 64 key blocks
NH = NLOC // 512       # 2 query halves of 512 (PSUM bank width)
ND = IN // P           # 4 d-chunks of x features


def _host_fallback(x, W0, W1, weight, weight_time):
    x = np.asarray(x, np.float32)
    q = x @ np.asarray(W0, np.float32).T
    k = np.asarray(np.asarray(W1, np.float32) @ x.T)        # [FEAT, N]
    out = np.empty((N, NOUT), np.float32)
    w = np.asarray(weight, np.float32)
    blk = 1024
    for i0 in range(0, N, blk):
        s = q[i0:i0 + blk] @ k                               # [blk, N]
        s -= s.max(1, keepdims=True)
        np.exp(s, out=s)
        s /= s.sum(1, keepdims=True)
        out[i0:i0 + blk] = ALPHA * ((s @ x) @ w)
    out += _time_branch(x) @ ((1.0 - ALPHA) * np.asarray(weight_time, np.float32))
    return out


def _time_branch(x):
    """G_time @ x computed exactly via prefix sums (Toeplitz structure)."""
    xf = np.asarray(x, np.float64)
    i = np.arange(N, dtype=np.float64)
    Pc = np.cumsum(xf, axis=0)                   # P_i = sum_{j<=i} x_j
    Qc = np.cumsum(i[:, None] * xf, axis=0)      # Q_i = sum_{j<=i} j*x_j
    Pn = Pc[-1]
    Qn = Qc[-1]
    A = 2.0 * (i[:, None] * Pc - Qc) + (Qn[None, :] - i[:, None] * Pn[None, :])
    S = N * N - (i * (i + 1) / 2 + (N - 1 - i) * (N - i) / 2)
    T = (N * Pn[None, :] - A) / S[:, None]
    return T.astype(np.float32)


def _install_wait_splitter():
    """This walrus build rejects instructions carrying more than ~2 sync
    waits. Rewrite the BIR right before compile: any instruction with >1
    on_wait keeps its last wait and gets the rest as standalone
    EventSemaphore instructions immediately before it on the same engine
    (same semantics: engine program order guarantees all waits complete
    before the instruction runs)."""
    from concourse import bass2jax
    if getattr(bass2jax, "_wait_split_installed", False):
        return
    import orjson

    def _split(bir):
        ctr = [0]

        def fix_block(b):
            out = []
            for inst in b.get("instructions", []):
                si = inst.get("sync_info")
                waits = (si or {}).get("on_wait") or []
                if len(waits) > 1 and inst.get("engine") not in (None, "Unassigned"):
                    extra, keep = waits[:-1], waits[-1:]
                    for w in extra:
                        ctr[0] += 1
                        out.append({
                            "debug": inst.get("debug"),
                            "engine": inst["engine"],
                            "ins": [], "outs": [],
                            "name": f"wsplit-{ctr[0]}",
                            "opcode": "EventSemaphore",
                            "sync_info": {"on_update": [], "on_wait": [w]},
                        })
                    si["on_wait"] = keep
                out.append(inst)
            b["instructions"] = out
            for sb in b.get("blocks", []):
                fix_block(sb)

        for fn in bir["functions"]:
            for b in fn["blocks"]:
                fix_block(b)
        return ctr[0]

    orig = bass2jax.compile_bir_kernel

    def wrapped(bir_json, tmpdir, neff_name="file.neff"):
        try:
            bir = orjson.loads(bir_json)
            n = _split(bir)
            if n:
                bir_json = orjson.dumps(bir)
        except Exception:
            traceback.print_exc()
        return orig(bir_json, tmpdir, neff_name)

    bass2jax.compile_bir_kernel = wrapped
    bass2jax._wait_split_installed = True


def _patched_tc(tile_mod, bass_mod):
    """TileContext whose tail drain splits its sem waits across one drain per
    proc -- this walrus build rejects >2 sync waits on a single CTRL inst."""
    from concourse.vector_clock import ScopedClock, VectorClock

    class PatchedTC(tile_mod.TileContext):
        def _drain_and_barrier(self, tick_clock, wait_clock):
            gc = tick_clock.global_clock
            n = len(gc)
            for p in range(n):
                t = gc[p]
                if t <= 0:
                    continue
                vec = [0] * n
                vec[p] = t
                d = self.nc.sync.drain()
                wait_clock.add_sem_waits(d.ins, ScopedClock({None: VectorClock(vec)}))
            self.nc.all_engine_barrier()
            popped = self.nc._tile_sem_poison_stack.pop()
            assert popped is self._sem_poison
            self.nc.clear_and_free_semaphores(list(self.sems.allocated().values()))
            self.nc.all_engine_barrier()

    return PatchedTC


def _build_nc(c_shift):
    from concourse import bass, tile, mybir
    from contextlib import ExitStack
    F32 = mybir.dt.float32
    BF16 = mybir.dt.bfloat16
    Exp = mybir.ActivationFunctionType.Exp
    ADD = mybir.AluOpType.add
    MUL = mybir.AluOpType.mult
    PatchedTC = _patched_tc(tile, bass)

    nc = bass.Bass("TRN2", num_devices=NCORES)
    qhiT = nc.declare_dram_parameter("qhiT", [FEAT, NLOC], BF16, isOutput=False)
    qloT = nc.declare_dram_parameter("qloT", [FEAT, NLOC], BF16, isOutput=False)
    khiT = nc.declare_dram_parameter("khiT", [FEAT, N], BF16, isOutput=False)
    kloT = nc.declare_dram_parameter("kloT", [FEAT, N], BF16, isOutput=False)
    xb = nc.declare_dram_parameter("xb", [N, IN], BF16, isOutput=False)
    trt = nc.declare_dram_parameter("trt", [IN, NLOC], BF16, isOutput=False)
    wb = nc.declare_dram_parameter("wb", [IN, NOUT], BF16, isOutput=False)
    wtb = nc.declare_dram_parameter("wtb", [IN, NOUT], BF16, isOutput=False)
    outT = nc.declare_dram_parameter("outT", [NOUT, NLOC], F32, isOutput=True)

    with PatchedTC(nc) as tc, ExitStack() as ctx:
        cst = ctx.enter_context(tc.tile_pool(name="cst", bufs=1))
        xpool = ctx.enter_context(tc.tile_pool(name="xp", bufs=1))
        epool = ctx.enter_context(tc.tile_pool(name="ep", bufs=4))
        upool = ctx.enter_context(tc.tile_pool(name="up", bufs=1, space="PSUM"))
        spool = ctx.enter_context(tc.tile_pool(name="sp", bufs=2, space="PSUM"))
        ppool = ctx.enter_context(tc.tile_pool(name="pp", bufs=2, space="PSUM"))
        usbp = ctx.enter_context(tc.tile_pool(name="usb", bufs=2))
        misc = ctx.enter_context(tc.tile_pool(name="misc", bufs=1))

        qh = cst.tile([FEAT, NLOC], BF16, name="qh")
        ql = cst.tile([FEAT, NLOC], BF16, name="ql")
        kh = cst.tile([FEAT, N], BF16, name="kh")
        kl = cst.tile([FEAT, N], BF16, name="kl")
        nc.sync.dma_start(qh[:], qhiT[:])
        nc.sync.dma_start(ql[:], qloT[:])
        nc.sync.dma_start(kh[:], khiT[:])
        nc.sync.dma_start(kl[:], kloT[:])

        xt = []
        for b in range(NBLK):
            t = xpool.tile([P, IN], BF16, name=f"x{b}", tag=f"x{b}")
            nc.sync.dma_start(t[:], xb[b * P:(b + 1) * P, :])
            xt.append(t)
        trtt = []
        for dd in range(ND):
            t = cst.tile([P, NLOC], BF16, name=f"tr{dd}", tag=f"tr{dd}")
            nc.sync.dma_start(t[:], trt[dd * P:(dd + 1) * P, :])
            trtt.append(t)
        wbt, wtbt = [], []
        for dd in range(ND):
            t = cst.tile([P, NOUT], BF16, name=f"wb{dd}", tag=f"wb{dd}")
            nc.sync.dma_start(t[:], wb[dd * P:(dd + 1) * P, :])
            wbt.append(t)
            t2 = cst.tile([P, NOUT], BF16, name=f"wt{dd}", tag=f"wt{dd}")
            nc.sync.dma_start(t2[:], wtb[dd * P:(dd + 1) * P, :])
            wtbt.append(t2)

        ones = misc.tile([P, 1], F32, name="ones")
        nc.vector.memset(ones[:], 1.0)
        bconst = misc.tile([P, 1], F32, name="bconst")
        nc.vector.memset(bconst[:], -float(c_shift))
        zacc = misc.tile([P, NLOC], F32, name="zacc")
        nc.vector.memset(zacc[:], 0.0)
        zsb = misc.tile([1, NLOC], F32, name="zsb")
        zrec = misc.tile([1, NLOC], F32, name="zrec")
        zrb = misc.tile([P, NLOC], F32, name="zrb")
        outsb = [misc.tile([P, NLOC], F32, name=f"ou{oo}", tag=f"ou{oo}")
                 for oo in range(ND)]

        for h in range(NH):
            msl = slice(h * 512, h * 512 + 512)
            ups = [upool.tile([P, 512], F32, name=f"u{h}_{dd}", tag=f"u{dd}")
                   for dd in range(ND)]
            for b in range(NBLK):
                jsl = slice(b * P, (b + 1) * P)
                sp = spool.tile([P, 512], F32, name=f"s{h}_{b}", tag="s")
                nc.tensor.matmul(sp[:], kh[:, jsl], qh[:, msl], start=True, stop=False)
                nc.tensor.matmul(sp[:], kh[:, jsl], ql[:, msl], start=False, stop=False)
                nc.tensor.matmul(sp[:], kl[:, jsl], qh[:, msl], start=False, stop=True)
                et = epool.tile([P, 512], BF16, name=f"e{h}_{b}", tag="e")
                nc.scalar.activation(et[:], sp[:], Exp, bias=bconst[:])
                nc.vector.tensor_tensor(zacc[:, msl], zacc[:, msl], et[:], ADD)
                for dd in range(ND):
                    dsl = slice(dd * P, (dd + 1) * P)
                    nc.tensor.matmul(ups[dd][:], xt[b][:, dsl], et[:],
                                     start=(b == 0), stop=(b == NBLK - 1))
            # Z for this half: partition-reduce via ones-matmul, then 1/Z
            zp = ppool.tile([P, 512], F32, name=f"zp{h}", tag="proj")
            nc.tensor.matmul(zp[0:1, :], ones[:], zacc[:, msl], start=True, stop=True)
            nc.vector.tensor_copy(zsb[0:1, msl], zp[0:1, :])
            nc.vector.reciprocal(zrec[0:1, msl], zsb[0:1, msl])
            nc.gpsimd.partition_broadcast(zrb[:, msl], zrec[0:1, msl])
            # scale U^T by 1/Z (frees the U PSUM banks), cast to bf16
            usb = []
            for dd in range(ND):
                t = usbp.tile([P, 512], BF16, name=f"us{h}_{dd}", tag=f"us{dd}")
                nc.vector.tensor_tensor(t[:], ups[dd][:], zrb[:, msl], MUL)
                usb.append(t)
            # fused projection: outT[o, m] = sum_d [wb;wtb][d,o] * [U/Z; trT][d,m]
            for oo in range(ND):
                osl = slice(oo * P, (oo + 1) * P)
                po = ppool.tile([P, 512], F32, name=f"po{h}_{oo}", tag="proj")
                for dd in range(ND):
                    nc.tensor.matmul(po[:], wbt[dd][:, osl], usb[dd][:],
                                     start=(dd == 0), stop=False)
                for dd in range(ND):
                    nc.tensor.matmul(po[:], wtbt[dd][:, osl], trtt[dd][:, msl],
                                     start=False, stop=(dd == ND - 1))
                nc.scalar.activation(outsb[oo][:, msl], po[:],
                                     mybir.ActivationFunctionType.Copy)
        for oo in range(ND):
            nc.sync.dma_start(outT[oo * P:(oo + 1) * P, :], outsb[oo][:])
    return nc


def _device_kernel(x, W0, W1, weight, weight_time):
    import ml_dtypes
    from concourse.bass_utils import run_bass_kernel_spmd
    _install_wait_splitter()

    bf = ml_dtypes.bfloat16
    x = np.asarray(x, np.float32)
    W0 = np.asarray(W0, np.float32)
    W1 = np.asarray(W1, np.float32)
    weight = np.asarray(weight, np.float32)
    weight_time = np.asarray(weight_time, np.float32)

    qT = np.ascontiguousarray((x @ W0.T).T)      # [FEAT, N] fp32
    kT = np.ascontiguousarray(W1 @ x.T)          # [FEAT, N] fp32

    def hilo(a):
        hi = a.astype(bf)
        lo = (a - hi.astype(np.float32)).astype(bf)
        return hi, lo

    khi, klo = hilo(kT)
    qhi, qlo = hilo(qT)
    xbf = x.astype(bf)

    # constant softmax shift: sampled row maxima + margin (fp32 exp has ~87 of
    # headroom on either side, so the sampling error margin is enormous)
    samp = qT[:, ::512].T @ kT                   # [16, N] scores
    c_shift = float(samp.max()) + 8.0

    trows = _time_branch(x)                      # exact G_time @ x, [N, IN]
    wbv = np.ascontiguousarray((ALPHA * weight).astype(bf))
    wtbv = np.ascontiguousarray(((1.0 - ALPHA) * weight_time).astype(bf))

    nc = _build_nc(c_shift)
    in_maps = []
    for c in range(NCORES):
        sl = slice(c * NLOC, (c + 1) * NLOC)
        in_maps.append(dict(
            qhiT=np.ascontiguousarray(qhi[:, sl]),
            qloT=np.ascontiguousarray(qlo[:, sl]),
            khiT=khi, kloT=klo, xb=xbf,
            trt=np.ascontiguousarray(trows[sl].T.astype(bf)),
            wb=wbv, wtb=wtbv,
        ))

    res = run_bass_kernel_spmd(nc, in_maps, list(range(NCORES)))
    out = np.empty((N, NOUT), np.float32)
    for c in range(NCORES):
        out[c * NLOC:(c + 1) * NLOC] = res.results[c]["outT"].T
    return out


def kernel(**inputs):
    try:
        out = _device_kernel(**inputs)
        if not np.isfinite(out).all():
            raise FloatingPointError("non-finite values in device output")
        return out.astype(np.asarray(inputs["x"]).dtype)
    except Exception:
        traceback.print_exc()
        sys.stderr.write("device path failed; using host fallback\n")
        return _host_fallback(**inputs)


# revision 10
# speedup vs baseline: 4.7737x; 4.7737x over previous
"""Trainium2 Bass kernel for nn_Graph_Layer_44787918963014 (gnn_message_passing).

out = ALPHA * softmax(q k^T) @ x @ weight + (1-ALPHA) * G_time @ x @ weight_time
with q = x@W0.T, k = x@W1.T, G_time the normalized (n-|i-j|) Toeplitz affinity.

Strategy (8 NeuronCores, row-sharded: core c owns queries [c*1024, (c+1)*1024)):
  host prep : q/k projections (cheap [N,512]@[512,128] GEMMs), bf16 hi+lo split
              (fp32-accurate scores from 3 bf16 matmuls); global constant score
              shift c (softmax-invariant, estimated from sampled rows, huge fp32
              margin); G_time @ x computed EXACTLY in O(N*D) via prefix sums
              (Toeplitz structure), so the time branch needs no N x N work.
  device    : per j-block of 128 keys -> scores S^T[j,m] in fp32 PSUM (3 bf16
              matmuls); exp(S^T - c) on ACT -> bf16 E^T; Z partials (DVE);
              U^T[d,m] += x_j^T E_j accumulated across all 64 j-blocks directly
              in PSUM (no SBUF flushes); Z partition-reduce via ones-matmul,
              reciprocal (DVE), partition-broadcast (GPSIMD); U^T scaled by 1/Z;
              single fused projection outT = [a*W; (1-a)*Wt]^T @ [U^T/Z; trT].
  host epi  : out[rows] = outT.T  (transpose only).

Self-contained: shapes hardcoded, no sibling imports. Falls back to an exact
blocked host computation if the device path fails for any reason.
"""
import sys, time, traceback
import numpy as np

N, IN, FEAT, NOUT = 8192, 512, 128, 512
ALPHA = 0.5
NCORES = 8
NLOC = N // NCORES     # 1024 queries per core
P = 128
NBLK = N // P          # 64 key blocks
NH = NLOC // 512       # 2 query halves of 512 (PSUM bank width)
ND = IN // P           # 4 d-chunks of x features


def _host_fallback(x, W0, W1, weight, weight_time):
    x = np.asarray(x, np.float32)
    q = x @ np.asarray(W0, np.float32).T
    k = np.asarray(np.asarray(W1, np.float32) @ x.T)        # [FEAT, N]
    out = np.empty((N, NOUT), np.float32)
    w = np.asarray(weight, np.float32)
    blk = 1024
    for i0 in range(0, N, blk):
        s = q[i0:i0 + blk] @ k                               # [blk, N]
        s -= s.max(1, keepdims=True)
        np.exp(s, out=s)
        s /= s.sum(1, keepdims=True)
        out[i0:i0 + blk] = ALPHA * ((s @ x) @ w)
    out += _time_branch(x) @ ((1.0 - ALPHA) * np.asarray(weight_time, np.float32))
    return out


def _time_branch(x):
    """G_time @ x computed exactly via prefix sums (Toeplitz structure)."""
    xf = np.asarray(x, np.float64)
    i = np.arange(N, dtype=np.float64)
    Pc = np.cumsum(xf, axis=0)                   # P_i = sum_{j<=i} x_j
    Qc = np.cumsum(i[:, None] * xf, axis=0)      # Q_i = sum_{j<=i} j*x_j
    Pn = Pc[-1]
    Qn = Qc[-1]
    A = 2.0 * (i[:, None] * Pc - Qc) + (Qn[None, :] - i[:, None] * Pn[None, :])
    S = N * N - (i * (i + 1) / 2 + (N - 1 - i) * (N - i) / 2)
    T = (N * Pn[None, :] - A) / S[:, None]
    return T.astype(np.float32)


def _install_wait_splitter():
    """This walrus build rejects instructions carrying more than ~2 sync
    waits. Rewrite the BIR right before compile: any instruction with >1
    on_wait keeps its last wait and gets the rest as standalone
    EventSemaphore instructions immediately before it on the same engine
    (same semantics: engine program order guarantees all waits complete
    before the instruction runs)."""
    from concourse import bass2jax
    if getattr(bass2jax, "_wait_split_installed", False):
        return
    import orjson

    def _split(bir):
        ctr = [0]

        def fix_block(b):
            out = []
            for inst in b.get("instructions", []):
                si = inst.get("sync_info")
                waits = (si or {}).get("on_wait") or []
                if len(waits) > 1 and inst.get("engine") not in (None, "Unassigned"):
                    extra, keep = waits[:-1], waits[-1:]
                    for w in extra:
                        ctr[0] += 1
                        out.append({
                            "debug": inst.get("debug"),
                            "engine": inst["engine"],
                            "ins": [], "outs": [],
                            "name": f"wsplit-{ctr[0]}",
                            "opcode": "EventSemaphore",
                            "sync_info": {"on_update": [], "on_wait": [w]},
                        })
                    si["on_wait"] = keep
                out.append(inst)
            b["instructions"] = out
            for sb in b.get("blocks", []):
                fix_block(sb)

        for fn in bir["functions"]:
            for b in fn["blocks"]:
                fix_block(b)
        return ctr[0]

    orig = bass2jax.compile_bir_kernel

    def wrapped(bir_json, tmpdir, neff_name="file.neff"):
        try:
            bir = orjson.loads(bir_json)
            n = _split(bir)
            if n:
                bir_json = orjson.dumps(bir)
        except Exception:
            traceback.print_exc()
        return orig(bir_json, tmpdir, neff_name)

    bass2jax.compile_bir_kernel = wrapped
    bass2jax._wait_split_installed = True


def _patched_tc(tile_mod, bass_mod):
    """TileContext whose tail drain splits its sem waits across one drain per
    proc -- this walrus build rejects >2 sync waits on a single CTRL inst."""
    from concourse.vector_clock import ScopedClock, VectorClock

    class PatchedTC(tile_mod.TileContext):
        def _drain_and_barrier(self, tick_clock, wait_clock):
            gc = tick_clock.global_clock
            n = len(gc)
            for p in range(n):
                t = gc[p]
                if t <= 0:
                    continue
                vec = [0] * n
                vec[p] = t
                d = self.nc.sync.drain()
                wait_clock.add_sem_waits(d.ins, ScopedClock({None: VectorClock(vec)}))
            self.nc.all_engine_barrier()
            popped = self.nc._tile_sem_poison_stack.pop()
            assert popped is self._sem_poison
            self.nc.clear_and_free_semaphores(list(self.sems.allocated().values()))
            self.nc.all_engine_barrier()

    return PatchedTC


def _build_nc(c_shift):
    from concourse import bass, tile, mybir
    from contextlib import ExitStack
    F32 = mybir.dt.float32
    BF16 = mybir.dt.bfloat16
    Exp = mybir.ActivationFunctionType.Exp
    ADD = mybir.AluOpType.add
    MUL = mybir.AluOpType.mult
    PatchedTC = _patched_tc(tile, bass)

    nc = bass.Bass("TRN2", num_devices=NCORES)
    qhiT = nc.declare_dram_parameter("qhiT", [FEAT, NLOC], BF16, isOutput=False)
    qloT = nc.declare_dram_parameter("qloT", [FEAT, NLOC], BF16, isOutput=False)
    khiT = nc.declare_dram_parameter("khiT", [FEAT, N], BF16, isOutput=False)
    kloT = nc.declare_dram_parameter("kloT", [FEAT, N], BF16, isOutput=False)
    xb = nc.declare_dram_parameter("xb", [N, IN], BF16, isOutput=False)
    trt = nc.declare_dram_parameter("trt", [IN, NLOC], BF16, isOutput=False)
    wb = nc.declare_dram_parameter("wb", [IN, NOUT], BF16, isOutput=False)
    wtb = nc.declare_dram_parameter("wtb", [IN, NOUT], BF16, isOutput=False)
    outT = nc.declare_dram_parameter("outT", [NOUT, NLOC], F32, isOutput=True)

    with PatchedTC(nc) as tc, ExitStack() as ctx:
        cst = ctx.enter_context(tc.tile_pool(name="cst", bufs=1))
        xpool = ctx.enter_context(tc.tile_pool(name="xp", bufs=1))
        epool = ctx.enter_context(tc.tile_pool(name="ep", bufs=4))
        upool = ctx.enter_context(tc.tile_pool(name="up", bufs=1, space="PSUM"))
        spool = ctx.enter_context(tc.tile_pool(name="sp", bufs=2, space="PSUM"))
        ppool = ctx.enter_context(tc.tile_pool(name="pp", bufs=2, space="PSUM"))
        usbp = ctx.enter_context(tc.tile_pool(name="usb", bufs=2))
        misc = ctx.enter_context(tc.tile_pool(name="misc", bufs=1))

        qh = cst.tile([FEAT, NLOC], BF16, name="qh")
        ql = cst.tile([FEAT, NLOC], BF16, name="ql")
        kh = cst.tile([FEAT, N], BF16, name="kh")
        kl = cst.tile([FEAT, N], BF16, name="kl")
        nc.sync.dma_start(qh[:], qhiT[:])
        nc.sync.dma_start(ql[:], qloT[:])
        nc.sync.dma_start(kh[:], khiT[:])
        nc.sync.dma_start(kl[:], kloT[:])

        xt = []
        for b in range(NBLK):
            t = xpool.tile([P, IN], BF16, name=f"x{b}", tag=f"x{b}")
            nc.sync.dma_start(t[:], xb[b * P:(b + 1) * P, :])
            xt.append(t)
        trtt = []
        for dd in range(ND):
            t = cst.tile([P, NLOC], BF16, name=f"tr{dd}", tag=f"tr{dd}")
            nc.sync.dma_start(t[:], trt[dd * P:(dd + 1) * P, :])
            trtt.append(t)
        wbt, wtbt = [], []
        for dd in range(ND):
            t = cst.tile([P, NOUT], BF16, name=f"wb{dd}", tag=f"wb{dd}")
            nc.sync.dma_start(t[:], wb[dd * P:(dd + 1) * P, :])
            wbt.append(t)
            t2 = cst.tile([P, NOUT], BF16, name=f"wt{dd}", tag=f"wt{dd}")
            nc.sync.dma_start(t2[:], wtb[dd * P:(dd + 1) * P, :])
            wtbt.append(t2)

        ones128 = misc.tile([P, P], F32, name="ones128")
        nc.vector.memset(ones128[:], 1.0)
        bconst = misc.tile([P, 1], F32, name="bconst")
        nc.vector.memset(bconst[:], -float(c_shift))
        zacc = misc.tile([P, NLOC], F32, name="zacc")
        nc.vector.memset(zacc[:], 0.0)
        outsb = [misc.tile([P, NLOC], F32, name=f"ou{oo}", tag=f"ou{oo}")
                 for oo in range(ND)]

        for h in range(NH):
            msl = slice(h * 512, h * 512 + 512)
            ups = [upool.tile([P, 512], F32, name=f"u{h}_{dd}", tag=f"u{dd}")
                   for dd in range(ND)]
            for b in range(NBLK):
                jsl = slice(b * P, (b + 1) * P)
                sp = spool.tile([P, 512], F32, name=f"s{h}_{b}", tag="s")
                nc.tensor.matmul(sp[:], kh[:, jsl], qh[:, msl], start=True, stop=False)
                nc.tensor.matmul(sp[:], kh[:, jsl], ql[:, msl], start=False, stop=False)
                nc.tensor.matmul(sp[:], kl[:, jsl], qh[:, msl], start=False, stop=True)
                et = epool.tile([P, 512], BF16, name=f"e{h}_{b}", tag="e")
                nc.scalar.activation(et[:], sp[:], Exp, bias=bconst[:])
                nc.vector.tensor_tensor(zacc[:, msl], zacc[:, msl], et[:], ADD)
                for dd in range(ND):
                    dsl = slice(dd * P, (dd + 1) * P)
                    nc.tensor.matmul(ups[dd][:], xt[b][:, dsl], et[:],
                                     start=(b == 0), stop=(b == NBLK - 1))
            # Z broadcast to all partitions in one matmul: (ones 128x128) @ zacc
            zps = ppool.tile([P, 512], F32, name=f"zp{h}", tag="proj")
            nc.tensor.matmul(zps[:], ones128[:], zacc[:, msl], start=True, stop=True)
            zrb = usbp.tile([P, 512], F32, name=f"zr{h}", tag="zr")
            nc.vector.reciprocal(zrb[:], zps[:])
            # scale U^T by 1/Z (frees the U PSUM banks), cast to bf16
            usb = []
            for dd in range(ND):
                t = usbp.tile([P, 512], BF16, name=f"us{h}_{dd}", tag=f"us{dd}")
                nc.vector.tensor_tensor(t[:], ups[dd][:], zrb[:], MUL)
                usb.append(t)
            # fused projection: outT[o, m] = sum_d [wb;wtb][d,o] * [U/Z; trT][d,m]
            for oo in range(ND):
                osl = slice(oo * P, (oo + 1) * P)
                po = ppool.tile([P, 512], F32, name=f"po{h}_{oo}", tag="proj")
                for dd in range(ND):
                    nc.tensor.matmul(po[:], wbt[dd][:, osl], usb[dd][:],
                                     start=(dd == 0), stop=False)
                for dd in range(ND):
                    nc.tensor.matmul(po[:], wtbt[dd][:, osl], trtt[dd][:, msl],
                                     start=False, stop=(dd == ND - 1))
                nc.scalar.activation(outsb[oo][:, msl], po[:],
                                     mybir.ActivationFunctionType.Copy)
        for oo in range(ND):
            nc.sync.dma_start(outT[oo * P:(oo + 1) * P, :], outsb[oo][:])
    return nc


def _device_kernel(x, W0, W1, weight, weight_time):
    import ml_dtypes
    from concourse.bass_utils import run_bass_kernel_spmd
    _install_wait_splitter()

    bf = ml_dtypes.bfloat16
    x = np.asarray(x, np.float32)
    W0 = np.asarray(W0, np.float32)
    W1 = np.asarray(W1, np.float32)
    weight = np.asarray(weight, np.float32)
    weight_time = np.asarray(weight_time, np.float32)

    qT = np.ascontiguousarray((x @ W0.T).T)      # [FEAT, N] fp32
    kT = np.ascontiguousarray(W1 @ x.T)          # [FEAT, N] fp32

    def hilo(a):
        hi = a.astype(bf)
        lo = (a - hi.astype(np.float32)).astype(bf)
        return hi, lo

    khi, klo = hilo(kT)
    qhi, qlo = hilo(qT)
    xbf = x.astype(bf)

    # constant softmax shift: sampled row maxima + margin (fp32 exp has ~87 of
    # headroom on either side, so the sampling error margin is enormous)
    samp = qT[:, ::512].T @ kT                   # [16, N] scores
    c_shift = float(samp.max()) + 8.0

    trows = _time_branch(x)                      # exact G_time @ x, [N, IN]
    wbv = np.ascontiguousarray((ALPHA * weight).astype(bf))
    wtbv = np.ascontiguousarray(((1.0 - ALPHA) * weight_time).astype(bf))

    nc = _build_nc(c_shift)
    in_maps = []
    for c in range(NCORES):
        sl = slice(c * NLOC, (c + 1) * NLOC)
        in_maps.append(dict(
            qhiT=np.ascontiguousarray(qhi[:, sl]),
            qloT=np.ascontiguousarray(qlo[:, sl]),
            khiT=khi, kloT=klo, xb=xbf,
            trt=np.ascontiguousarray(trows[sl].T.astype(bf)),
            wb=wbv, wtb=wtbv,
        ))

    res = run_bass_kernel_spmd(nc, in_maps, list(range(NCORES)))
    out = np.empty((N, NOUT), np.float32)
    for c in range(NCORES):
        out[c * NLOC:(c + 1) * NLOC] = res.results[c]["outT"].T
    return out


def kernel(**inputs):
    try:
        out = _device_kernel(**inputs)
        if not np.isfinite(out).all():
            raise FloatingPointError("non-finite values in device output")
        return out.astype(np.asarray(inputs["x"]).dtype)
    except Exception:
        traceback.print_exc()
        sys.stderr.write("device path failed; using host fallback\n")
        return _host_fallback(**inputs)


# revision 13
# speedup vs baseline: 5.4188x; 1.1351x over previous
"""Trainium2 Bass kernel for nn_Graph_Layer_44787918963014 (gnn_message_passing).

out = ALPHA * softmax(q k^T) @ x @ weight + (1-ALPHA) * G_time @ x @ weight_time
with q = x@W0.T, k = x@W1.T, G_time the normalized (n-|i-j|) Toeplitz affinity.

Strategy (8 NeuronCores, row-sharded: core c owns queries [c*1024, (c+1)*1024)):
  host prep : q/k projections (cheap [N,512]@[512,128] GEMMs), bf16 hi+lo split
              (fp32-accurate scores from 3 bf16 matmuls); global constant score
              shift c (softmax-invariant, estimated from sampled rows, huge fp32
              margin); G_time @ x computed EXACTLY in O(N*D) via prefix sums
              (Toeplitz structure), so the time branch needs no N x N work.
  device    : per j-block of 128 keys -> scores S^T[j,m] in fp32 PSUM (3 bf16
              matmuls); exp(S^T - c) on ACT -> bf16 E^T; Z partials (DVE);
              U^T[d,m] += x_j^T E_j accumulated across all 64 j-blocks directly
              in PSUM (no SBUF flushes); Z partition-reduce via ones-matmul,
              reciprocal (DVE), partition-broadcast (GPSIMD); U^T scaled by 1/Z;
              single fused projection outT = [a*W; (1-a)*Wt]^T @ [U^T/Z; trT].
  host epi  : out[rows] = outT.T  (transpose only).

Self-contained: shapes hardcoded, no sibling imports. Falls back to an exact
blocked host computation if the device path fails for any reason.
"""
import sys, time, traceback
import numpy as np

N, IN, FEAT, NOUT = 8192, 512, 128, 512
ALPHA = 0.5
NCORES = 8
NLOC = N // NCORES     # 1024 queries per core
P = 128
NBLK = N // P          # 64 key blocks
NH = NLOC // 512       # 2 query halves of 512 (PSUM bank width)
ND = IN // P           # 4 d-chunks of x features


def _host_fallback(x, W0, W1, weight, weight_time):
    x = np.asarray(x, np.float32)
    q = x @ np.asarray(W0, np.float32).T
    k = np.asarray(np.asarray(W1, np.float32) @ x.T)        # [FEAT, N]
    out = np.empty((N, NOUT), np.float32)
    w = np.asarray(weight, np.float32)
    blk = 1024
    for i0 in range(0, N, blk):
        s = q[i0:i0 + blk] @ k                               # [blk, N]
        s -= s.max(1, keepdims=True)
        np.exp(s, out=s)
        s /= s.sum(1, keepdims=True)
        out[i0:i0 + blk] = ALPHA * ((s @ x) @ w)
    out += _time_branch(x) @ ((1.0 - ALPHA) * np.asarray(weight_time, np.float32))
    return out


def _time_branch(x):
    """G_time @ x computed exactly via prefix sums (Toeplitz structure)."""
    xf = np.asarray(x, np.float64)
    i = np.arange(N, dtype=np.float64)
    Pc = np.cumsum(xf, axis=0)                   # P_i = sum_{j<=i} x_j
    Qc = np.cumsum(i[:, None] * xf, axis=0)      # Q_i = sum_{j<=i} j*x_j
    Pn = Pc[-1]
    Qn = Qc[-1]
    A = 2.0 * (i[:, None] * Pc - Qc) + (Qn[None, :] - i[:, None] * Pn[None, :])
    S = N * N - (i * (i + 1) / 2 + (N - 1 - i) * (N - i) / 2)
    T = (N * Pn[None, :] - A) / S[:, None]
    return T.astype(np.float32)


def _install_wait_splitter():
    """This walrus build rejects instructions carrying more than ~2 sync
    waits. Rewrite the BIR right before compile: any instruction with >1
    on_wait keeps its last wait and gets the rest as standalone
    EventSemaphore instructions immediately before it on the same engine
    (same semantics: engine program order guarantees all waits complete
    before the instruction runs)."""
    from concourse import bass2jax
    if getattr(bass2jax, "_wait_split_installed", False):
        return
    import orjson

    def _split(bir):
        ctr = [0]

        def fix_block(b):
            out = []
            for inst in b.get("instructions", []):
                si = inst.get("sync_info")
                waits = (si or {}).get("on_wait") or []
                if len(waits) > 1 and inst.get("engine") not in (None, "Unassigned"):
                    extra, keep = waits[:-1], waits[-1:]
                    for w in extra:
                        ctr[0] += 1
                        out.append({
                            "debug": inst.get("debug"),
                            "engine": inst["engine"],
                            "ins": [], "outs": [],
                            "name": f"wsplit-{ctr[0]}",
                            "opcode": "EventSemaphore",
                            "sync_info": {"on_update": [], "on_wait": [w]},
                        })
                    si["on_wait"] = keep
                out.append(inst)
            b["instructions"] = out
            for sb in b.get("blocks", []):
                fix_block(sb)

        for fn in bir["functions"]:
            for b in fn["blocks"]:
                fix_block(b)
        return ctr[0]

    orig = bass2jax.compile_bir_kernel

    def wrapped(bir_json, tmpdir, neff_name="file.neff"):
        try:
            bir = orjson.loads(bir_json)
            n = _split(bir)
            if n:
                bir_json = orjson.dumps(bir)
        except Exception:
            traceback.print_exc()
        return orig(bir_json, tmpdir, neff_name)

    bass2jax.compile_bir_kernel = wrapped
    bass2jax._wait_split_installed = True


def _patched_tc(tile_mod, bass_mod):
    """TileContext whose tail drain splits its sem waits across one drain per
    proc -- this walrus build rejects >2 sync waits on a single CTRL inst."""
    from concourse.vector_clock import ScopedClock, VectorClock

    class PatchedTC(tile_mod.TileContext):
        def _drain_and_barrier(self, tick_clock, wait_clock):
            gc = tick_clock.global_clock
            n = len(gc)
            for p in range(n):
                t = gc[p]
                if t <= 0:
                    continue
                vec = [0] * n
                vec[p] = t
                d = self.nc.sync.drain()
                wait_clock.add_sem_waits(d.ins, ScopedClock({None: VectorClock(vec)}))
            self.nc.all_engine_barrier()
            popped = self.nc._tile_sem_poison_stack.pop()
            assert popped is self._sem_poison
            self.nc.clear_and_free_semaphores(list(self.sems.allocated().values()))
            self.nc.all_engine_barrier()

    return PatchedTC


def _build_nc(c_shift):
    from concourse import bass, tile, mybir
    from contextlib import ExitStack
    F32 = mybir.dt.float32
    BF16 = mybir.dt.bfloat16
    Exp = mybir.ActivationFunctionType.Exp
    ADD = mybir.AluOpType.add
    MUL = mybir.AluOpType.mult
    PatchedTC = _patched_tc(tile, bass)

    nc = bass.Bass("TRN2", num_devices=NCORES)
    qhiT = nc.declare_dram_parameter("qhiT", [FEAT, NLOC], BF16, isOutput=False)
    qloT = nc.declare_dram_parameter("qloT", [FEAT, NLOC], BF16, isOutput=False)
    khiT = nc.declare_dram_parameter("khiT", [FEAT, N], BF16, isOutput=False)
    kloT = nc.declare_dram_parameter("kloT", [FEAT, N], BF16, isOutput=False)
    xb = nc.declare_dram_parameter("xb", [N, IN], BF16, isOutput=False)
    trt = nc.declare_dram_parameter("trt", [IN, NLOC], BF16, isOutput=False)
    wb = nc.declare_dram_parameter("wb", [IN, NOUT], BF16, isOutput=False)
    wtb = nc.declare_dram_parameter("wtb", [IN, NOUT], BF16, isOutput=False)
    outT = nc.declare_dram_parameter("outT", [NOUT, NLOC], F32, isOutput=True)

    with PatchedTC(nc) as tc, ExitStack() as ctx:
        cst = ctx.enter_context(tc.tile_pool(name="cst", bufs=1))
        xpool = ctx.enter_context(tc.tile_pool(name="xp", bufs=1))
        epool = ctx.enter_context(tc.tile_pool(name="ep", bufs=4))
        upool = ctx.enter_context(tc.tile_pool(name="up", bufs=1, space="PSUM"))
        spool = ctx.enter_context(tc.tile_pool(name="sp", bufs=2, space="PSUM"))
        ppool = ctx.enter_context(tc.tile_pool(name="pp", bufs=2, space="PSUM"))
        usbp = ctx.enter_context(tc.tile_pool(name="usb", bufs=2))
        misc = ctx.enter_context(tc.tile_pool(name="misc", bufs=1))

        qh = cst.tile([FEAT, NLOC], BF16, name="qh")
        ql = cst.tile([FEAT, NLOC], BF16, name="ql")
        kh = cst.tile([FEAT, N], BF16, name="kh")
        kl = cst.tile([FEAT, N], BF16, name="kl")
        nc.sync.dma_start(qh[:], qhiT[:])
        nc.sync.dma_start(ql[:], qloT[:])
        nc.sync.dma_start(kh[:], khiT[:])
        nc.sync.dma_start(kl[:], kloT[:])

        xt = []
        for b in range(NBLK):
            t = xpool.tile([P, IN], BF16, name=f"x{b}", tag=f"x{b}")
            nc.sync.dma_start(t[:], xb[b * P:(b + 1) * P, :])
            xt.append(t)
        trtt = []
        for dd in range(ND):
            t = cst.tile([P, NLOC], BF16, name=f"tr{dd}", tag=f"tr{dd}")
            nc.sync.dma_start(t[:], trt[dd * P:(dd + 1) * P, :])
            trtt.append(t)
        wbt, wtbt = [], []
        for dd in range(ND):
            t = cst.tile([P, NOUT], BF16, name=f"wb{dd}", tag=f"wb{dd}")
            nc.sync.dma_start(t[:], wb[dd * P:(dd + 1) * P, :])
            wbt.append(t)
            t2 = cst.tile([P, NOUT], BF16, name=f"wt{dd}", tag=f"wt{dd}")
            nc.sync.dma_start(t2[:], wtb[dd * P:(dd + 1) * P, :])
            wtbt.append(t2)

        ones128 = misc.tile([P, P], F32, name="ones128")
        nc.vector.memset(ones128[:], 1.0)
        bconst = misc.tile([P, 1], F32, name="bconst")
        nc.vector.memset(bconst[:], -float(c_shift))
        zacc = misc.tile([P, NLOC], F32, name="zacc")
        nc.vector.memset(zacc[:], 0.0)
        outsb = [misc.tile([P, NLOC], F32, name=f"ou{oo}", tag=f"ou{oo}")
                 for oo in range(ND)]

        for h in range(NH):
            msl = slice(h * 512, h * 512 + 512)
            ups = [upool.tile([P, 512], F32, name=f"u{h}_{dd}", tag=f"u{dd}")
                   for dd in range(ND)]
            for b in range(NBLK):
                jsl = slice(b * P, (b + 1) * P)
                sp = spool.tile([P, 512], F32, name=f"s{h}_{b}", tag="s")
                nc.tensor.matmul(sp[:], kh[:, jsl], qh[:, msl], start=True, stop=False)
                nc.tensor.matmul(sp[:], kh[:, jsl], ql[:, msl], start=False, stop=False)
                nc.tensor.matmul(sp[:], kl[:, jsl], qh[:, msl], start=False, stop=True)
                et = epool.tile([P, 512], BF16, name=f"e{h}_{b}", tag="e")
                nc.scalar.activation(et[:], sp[:], Exp, bias=bconst[:])
                nc.vector.tensor_tensor(zacc[:, msl], zacc[:, msl], et[:], ADD)
                for dd in range(ND):
                    dsl = slice(dd * P, (dd + 1) * P)
                    nc.tensor.matmul(ups[dd][:], xt[b][:, dsl], et[:],
                                     start=(b == 0), stop=(b == NBLK - 1))
            # Z broadcast to all partitions in one matmul: (ones 128x128) @ zacc
            zps = ppool.tile([P, 512], F32, name=f"zp{h}", tag="proj")
            nc.tensor.matmul(zps[:], ones128[:], zacc[:, msl], start=True, stop=True)
            zrb = usbp.tile([P, 512], F32, name=f"zr{h}", tag="zr")
            nc.vector.reciprocal(zrb[:], zps[:])
            # scale U^T by 1/Z (frees the U PSUM banks), cast to bf16
            usb = []
            for dd in range(ND):
                t = usbp.tile([P, 512], BF16, name=f"us{h}_{dd}", tag=f"us{dd}")
                nc.vector.tensor_tensor(t[:], ups[dd][:], zrb[:], MUL)
                usb.append(t)
            # fused projection: outT[o, m] = sum_d [wb;wtb][d,o] * [U/Z; trT][d,m]
            for oo in range(ND):
                osl = slice(oo * P, (oo + 1) * P)
                po = ppool.tile([P, 512], F32, name=f"po{h}_{oo}", tag="proj")
                for dd in range(ND):
                    nc.tensor.matmul(po[:], wbt[dd][:, osl], usb[dd][:],
                                     start=(dd == 0), stop=False)
                for dd in range(ND):
                    nc.tensor.matmul(po[:], wtbt[dd][:, osl], trtt[dd][:, msl],
                                     start=False, stop=(dd == ND - 1))
                nc.scalar.activation(outsb[oo][:, msl], po[:],
                                     mybir.ActivationFunctionType.Copy)
        for oo in range(ND):
            nc.sync.dma_start(outT[oo * P:(oo + 1) * P, :], outsb[oo][:])
    return nc


def _device_kernel(x, W0, W1, weight, weight_time):
    import ml_dtypes
    from concourse.bass_utils import run_bass_kernel_spmd
    _install_wait_splitter()
    _t0 = time.time()

    def _mark(m):
        sys.stderr.write(f"[ktiming] {m}: {time.time()-_t0:.2f}s\n")
        sys.stderr.flush()

    bf = ml_dtypes.bfloat16
    x = np.asarray(x, np.float32)
    W0 = np.asarray(W0, np.float32)
    W1 = np.asarray(W1, np.float32)
    weight = np.asarray(weight, np.float32)
    weight_time = np.asarray(weight_time, np.float32)

    qT = np.ascontiguousarray((x @ W0.T).T)      # [FEAT, N] fp32
    kT = np.ascontiguousarray(W1 @ x.T)          # [FEAT, N] fp32

    def hilo(a):
        hi = a.astype(bf)
        lo = (a - hi.astype(np.float32)).astype(bf)
        return hi, lo

    khi, klo = hilo(kT)
    qhi, qlo = hilo(qT)
    xbf = x.astype(bf)

    # constant softmax shift: sampled row maxima + margin (fp32 exp has ~87 of
    # headroom on either side, so the sampling error margin is enormous)
    samp = qT[:, ::512].T @ kT                   # [16, N] scores
    c_shift = float(samp.max()) + 8.0

    trows = _time_branch(x)                      # exact G_time @ x, [N, IN]
    wbv = np.ascontiguousarray((ALPHA * weight).astype(bf))
    wtbv = np.ascontiguousarray(((1.0 - ALPHA) * weight_time).astype(bf))
    _mark("host prep")

    nc = _build_nc(c_shift)
    _mark("build+schedule")
    in_maps = []
    for c in range(NCORES):
        sl = slice(c * NLOC, (c + 1) * NLOC)
        in_maps.append(dict(
            qhiT=np.ascontiguousarray(qhi[:, sl]),
            qloT=np.ascontiguousarray(qlo[:, sl]),
            khiT=khi, kloT=klo, xb=xbf,
            trt=np.ascontiguousarray(trows[sl].T.astype(bf)),
            wb=wbv, wtb=wtbv,
        ))

    _mark("in_maps")
    res = run_bass_kernel_spmd(nc, in_maps, list(range(NCORES)))
    _mark("device run (compile+ship+exec)")
    out = np.empty((N, NOUT), np.float32)
    for c in range(NCORES):
        out[c * NLOC:(c + 1) * NLOC] = res.results[c]["outT"].T
    _mark("epilogue")
    return out


def kernel(**inputs):
    try:
        out = _device_kernel(**inputs)
        if not np.isfinite(out).all():
            raise FloatingPointError("non-finite values in device output")
        return out.astype(np.asarray(inputs["x"]).dtype)
    except Exception:
        traceback.print_exc()
        sys.stderr.write("device path failed; using host fallback\n")
        return _host_fallback(**inputs)


# revision 14
# speedup vs baseline: 5.4650x; 1.0085x over previous
"""Trainium2 Bass kernel for nn_Graph_Layer_44787918963014 (gnn_message_passing).

out = ALPHA * softmax(q k^T) @ x @ weight + (1-ALPHA) * G_time @ x @ weight_time
with q = x@W0.T, k = x@W1.T, G_time the normalized (n-|i-j|) Toeplitz affinity.

Strategy (8 NeuronCores, row-sharded: core c owns queries [c*1024, (c+1)*1024)):
  host prep : q/k projections (cheap [N,512]@[512,128] GEMMs, shipped fp32);
              global constant score shift c (softmax-invariant, estimated from
              sampled rows; fp32 exp has ~87 of headroom each side);
              G_time @ x computed EXACTLY in O(N*D) via prefix sums (Toeplitz
              structure), so the time branch needs no N x N work.
  device    : per j-block of 128 keys -> scores S^T[j,m] via one fp32 matmul
              into PSUM; exp(S^T - c) on ACT -> bf16 E^T; Z partials (DVE);
              U^T[d,m] += x_j^T E_j accumulated across all 64 j-blocks directly
              in PSUM (no SBUF flushes); Z partition-broadcast via all-ones
              matmul; 1/Z on DVE; single fused projection
              outT = [a*W; (1-a)*Wt]^T @ [U^T/Z; trT].
  host epi  : out[rows] = outT.T  (transpose only).

Self-contained: shapes hardcoded, no sibling imports. Falls back to an exact
blocked host computation if the device path fails for any reason.
"""
import os, sys, time, hashlib, traceback
import numpy as np

N, IN, FEAT, NOUT = 8192, 512, 128, 512
ALPHA = 0.5
NCORES = 8
NLOC = N // NCORES     # 1024 queries per core
P = 128
NBLK = N // P          # 64 key blocks
NH = NLOC // 512       # 2 query halves of 512 (PSUM bank width)
ND = IN // P           # 4 d-chunks of x features

_NEFF_CACHE_DIR = "/tmp/.bass_neff_cache"

try:
    import ml_dtypes
    from contextlib import ExitStack
    from concourse import bass, tile, mybir, bass2jax
    from concourse.bass_utils import run_bass_kernel_spmd
    from concourse.vector_clock import ScopedClock, VectorClock
    _IMPORTS_OK = True
except Exception:
    traceback.print_exc()
    _IMPORTS_OK = False


def _host_fallback(x, W0, W1, weight, weight_time):
    x = np.asarray(x, np.float32)
    q = x @ np.asarray(W0, np.float32).T
    k = np.asarray(np.asarray(W1, np.float32) @ x.T)        # [FEAT, N]
    out = np.empty((N, NOUT), np.float32)
    w = np.asarray(weight, np.float32)
    blk = 1024
    for i0 in range(0, N, blk):
        s = q[i0:i0 + blk] @ k                               # [blk, N]
        s -= s.max(1, keepdims=True)
        np.exp(s, out=s)
        s /= s.sum(1, keepdims=True)
        out[i0:i0 + blk] = ALPHA * ((s @ x) @ w)
    out += _time_branch(x) @ ((1.0 - ALPHA) * np.asarray(weight_time, np.float32))
    return out


def _time_branch(x):
    """G_time @ x computed exactly via prefix sums (Toeplitz structure).
    fp32 cumsums: partial sums stay O(300), so the error reaching T after the
    /S_i (~5e7) normalization is ~1e-7 -- far below the bf16 shipping dtype."""
    xf = np.asarray(x, np.float32)
    i = np.arange(N, dtype=np.float32)
    i64 = np.arange(N, dtype=np.float64)
    Pc = np.cumsum(xf, axis=0)                   # P_i = sum_{j<=i} x_j
    Qc = np.cumsum(i[:, None] * xf, axis=0)      # Q_i = sum_{j<=i} j*x_j
    Pn = Pc[-1].astype(np.float64)
    Qn = Qc[-1].astype(np.float64)
    A = 2.0 * (i[:, None] * Pc - Qc) + (Qn[None, :] - i64[:, None] * Pn[None, :])
    S = N * N - (i64 * (i64 + 1) / 2 + (N - 1 - i64) * (N - i64) / 2)
    T = (N * Pn[None, :] - A) / S[:, None]
    return T.astype(np.float32)


def _install_compile_shims():
    """Two shims around the BIR -> NEFF compile:
    1. Wait splitter: this walrus build rejects instructions carrying more
       than ~2 sync waits. Any instruction with >1 on_wait keeps its last
       wait; the rest become standalone EventSemaphore instructions
       immediately before it on the same engine (identical semantics:
       engine program order runs them first).
    2. NEFF disk cache keyed on the (rewritten) BIR bytes, so repeat
       invocations in fresh processes skip walrus entirely.
    """
    if getattr(bass2jax, "_compile_shims_installed", False):
        return
    import orjson

    def _split(bir):
        ctr = [0]

        def fix_block(b):
            out = []
            for inst in b.get("instructions", []):
                si = inst.get("sync_info")
                waits = (si or {}).get("on_wait") or []
                if len(waits) > 1 and inst.get("engine") not in (None, "Unassigned"):
                    extra, keep = waits[:-1], waits[-1:]
                    for w in extra:
                        ctr[0] += 1
                        out.append({
                            "debug": inst.get("debug"),
                            "engine": inst["engine"],
                            "ins": [], "outs": [],
                            "name": f"wsplit-{ctr[0]}",
                            "opcode": "EventSemaphore",
                            "sync_info": {"on_update": [], "on_wait": [w]},
                        })
                    si["on_wait"] = keep
                out.append(inst)
            b["instructions"] = out
            for sb in b.get("blocks", []):
                fix_block(sb)

        for fn in bir["functions"]:
            for b in fn["blocks"]:
                fix_block(b)
        return ctr[0]

    orig = bass2jax.compile_bir_kernel

    def wrapped(bir_json, tmpdir, neff_name="file.neff"):
        try:
            bir = orjson.loads(bir_json)
            if _split(bir):
                bir_json = orjson.dumps(bir)
        except Exception:
            traceback.print_exc()
        cache_path = None
        try:
            os.makedirs(_NEFF_CACHE_DIR, exist_ok=True)
            key = hashlib.sha256(bir_json).hexdigest()
            cache_path = os.path.join(_NEFF_CACHE_DIR, f"{key}.neff")
            if os.path.exists(cache_path):
                dst = os.path.join(tmpdir, neff_name)
                with open(cache_path, "rb") as f, open(dst, "wb") as g:
                    g.write(f.read())
                sys.stderr.write("[ktiming] neff cache hit\n")
                return dst
        except Exception:
            traceback.print_exc()
        t = time.time()
        neff_path = orig(bir_json, tmpdir, neff_name)
        sys.stderr.write(f"[ktiming] walrus compile: {time.time()-t:.2f}s\n")
        try:
            if cache_path:
                tmp = cache_path + ".tmp"
                with open(neff_path, "rb") as f, open(tmp, "wb") as g:
                    g.write(f.read())
                os.replace(tmp, cache_path)
        except Exception:
            traceback.print_exc()
        return neff_path

    bass2jax.compile_bir_kernel = wrapped
    bass2jax._compile_shims_installed = True


class _PatchedTC(tile.TileContext if _IMPORTS_OK else object):
    """Tail drain emits one drain per proc (>2 sync waits on one CTRL inst
    is rejected by this walrus build)."""

    def _drain_and_barrier(self, tick_clock, wait_clock):
        gc = tick_clock.global_clock
        n = len(gc)
        for p in range(n):
            t = gc[p]
            if t <= 0:
                continue
            vec = [0] * n
            vec[p] = t
            d = self.nc.sync.drain()
            wait_clock.add_sem_waits(d.ins, ScopedClock({None: VectorClock(vec)}))
        self.nc.all_engine_barrier()
        popped = self.nc._tile_sem_poison_stack.pop()
        assert popped is self._sem_poison
        self.nc.clear_and_free_semaphores(list(self.sems.allocated().values()))
        self.nc.all_engine_barrier()


def _build_nc(c_shift):
    F32 = mybir.dt.float32
    BF16 = mybir.dt.bfloat16
    Exp = mybir.ActivationFunctionType.Exp
    ADD = mybir.AluOpType.add
    MUL = mybir.AluOpType.mult

    nc = bass.Bass("TRN2", num_devices=NCORES)
    qT = nc.declare_dram_parameter("qT", [FEAT, NLOC], F32, isOutput=False)
    kT = nc.declare_dram_parameter("kT", [FEAT, N], F32, isOutput=False)
    xb = nc.declare_dram_parameter("xb", [N, IN], BF16, isOutput=False)
    trt = nc.declare_dram_parameter("trt", [IN, NLOC], BF16, isOutput=False)
    wb = nc.declare_dram_parameter("wb", [IN, NOUT], BF16, isOutput=False)
    wtb = nc.declare_dram_parameter("wtb", [IN, NOUT], BF16, isOutput=False)
    outT = nc.declare_dram_parameter("outT", [NOUT, NLOC], F32, isOutput=True)

    with _PatchedTC(nc) as tc, ExitStack() as ctx:
        cst = ctx.enter_context(tc.tile_pool(name="cst", bufs=1))
        xpool = ctx.enter_context(tc.tile_pool(name="xp", bufs=1))
        epool = ctx.enter_context(tc.tile_pool(name="ep", bufs=4))
        upool = ctx.enter_context(tc.tile_pool(name="up", bufs=1, space="PSUM"))
        spool = ctx.enter_context(tc.tile_pool(name="sp", bufs=2, space="PSUM"))
        ppool = ctx.enter_context(tc.tile_pool(name="pp", bufs=2, space="PSUM"))
        usbp = ctx.enter_context(tc.tile_pool(name="usb", bufs=2))
        misc = ctx.enter_context(tc.tile_pool(name="misc", bufs=1))

        qt = cst.tile([FEAT, NLOC], F32, name="qt")
        kt = cst.tile([FEAT, N], F32, name="kt")
        nc.sync.dma_start(qt[:], qT[:])
        nc.sync.dma_start(kt[:], kT[:])

        xt = []
        for b in range(NBLK):
            t = xpool.tile([P, IN], BF16, name=f"x{b}", tag=f"x{b}")
            nc.sync.dma_start(t[:], xb[b * P:(b + 1) * P, :])
            xt.append(t)
        trtt = []
        for dd in range(ND):
            t = cst.tile([P, NLOC], BF16, name=f"tr{dd}", tag=f"tr{dd}")
            nc.sync.dma_start(t[:], trt[dd * P:(dd + 1) * P, :])
            trtt.append(t)
        wbt, wtbt = [], []
        for dd in range(ND):
            t = cst.tile([P, NOUT], BF16, name=f"wb{dd}", tag=f"wb{dd}")
            nc.sync.dma_start(t[:], wb[dd * P:(dd + 1) * P, :])
            wbt.append(t)
            t2 = cst.tile([P, NOUT], BF16, name=f"wt{dd}", tag=f"wt{dd}")
            nc.sync.dma_start(t2[:], wtb[dd * P:(dd + 1) * P, :])
            wtbt.append(t2)

        ones128 = misc.tile([P, P], F32, name="ones128")
        nc.vector.memset(ones128[:], 1.0)
        bconst = misc.tile([P, 1], F32, name="bconst")
        nc.vector.memset(bconst[:], -float(c_shift))
        zacc = misc.tile([P, NLOC], F32, name="zacc")
        nc.vector.memset(zacc[:], 0.0)
        outsb = [misc.tile([P, NLOC], F32, name=f"ou{oo}", tag=f"ou{oo}")
                 for oo in range(ND)]

        for h in range(NH):
            msl = slice(h * 512, h * 512 + 512)
            ups = [upool.tile([P, 512], F32, name=f"u{h}_{dd}", tag=f"u{dd}")
                   for dd in range(ND)]
            for b in range(NBLK):
                jsl = slice(b * P, (b + 1) * P)
                sp = spool.tile([P, 512], F32, name=f"s{h}_{b}", tag="s")
                nc.tensor.matmul(sp[:], kt[:, jsl], qt[:, msl], start=True, stop=True)
                et = epool.tile([P, 512], BF16, name=f"e{h}_{b}", tag="e")
                nc.scalar.activation(et[:], sp[:], Exp, bias=bconst[:])
                nc.vector.tensor_tensor(zacc[:, msl], zacc[:, msl], et[:], ADD)
                for dd in range(ND):
                    dsl = slice(dd * P, (dd + 1) * P)
                    nc.tensor.matmul(ups[dd][:], xt[b][:, dsl], et[:],
                                     start=(b == 0), stop=(b == NBLK - 1))
            # Z broadcast to all partitions in one matmul: (ones 128x128) @ zacc
            zps = ppool.tile([P, 512], F32, name=f"zp{h}", tag="proj")
            nc.tensor.matmul(zps[:], ones128[:], zacc[:, msl], start=True, stop=True)
            zrb = usbp.tile([P, 512], F32, name=f"zr{h}", tag="zr")
            nc.vector.reciprocal(zrb[:], zps[:])
            # scale U^T by 1/Z (frees the U PSUM banks), cast to bf16
            usb = []
            for dd in range(ND):
                t = usbp.tile([P, 512], BF16, name=f"us{h}_{dd}", tag=f"us{dd}")
                nc.vector.tensor_tensor(t[:], ups[dd][:], zrb[:], MUL)
                usb.append(t)
            # fused projection: outT[o, m] = sum_d [wb;wtb][d,o] * [U/Z; trT][d,m]
            for oo in range(ND):
                osl = slice(oo * P, (oo + 1) * P)
                po = ppool.tile([P, 512], F32, name=f"po{h}_{oo}", tag="proj")
                for dd in range(ND):
                    nc.tensor.matmul(po[:], wbt[dd][:, osl], usb[dd][:],
                                     start=(dd == 0), stop=False)
                for dd in range(ND):
                    nc.tensor.matmul(po[:], wtbt[dd][:, osl], trtt[dd][:, msl],
                                     start=False, stop=(dd == ND - 1))
                nc.scalar.activation(outsb[oo][:, msl], po[:],
                                     mybir.ActivationFunctionType.Copy)
        for oo in range(ND):
            nc.sync.dma_start(outT[oo * P:(oo + 1) * P, :], outsb[oo][:])
    return nc


def _device_kernel(x, W0, W1, weight, weight_time):
    _install_compile_shims()
    _t0 = time.time()

    def _mark(m):
        sys.stderr.write(f"[ktiming] {m}: {time.time()-_t0:.2f}s\n")
        sys.stderr.flush()

    bf = ml_dtypes.bfloat16
    x = np.asarray(x, np.float32)
    W0 = np.asarray(W0, np.float32)
    W1 = np.asarray(W1, np.float32)
    weight = np.asarray(weight, np.float32)
    weight_time = np.asarray(weight_time, np.float32)

    qTv = np.ascontiguousarray((x @ W0.T).T)     # [FEAT, N] fp32
    kTv = np.ascontiguousarray(W1 @ x.T)         # [FEAT, N] fp32
    xbf = x.astype(bf)

    # constant softmax shift: sampled row maxima + margin (fp32 exp has ~87 of
    # headroom on either side, so the sampling error margin is enormous)
    samp = qTv[:, ::512].T @ kTv                 # [16, N] scores
    c_shift = float(samp.max()) + 8.0

    trows = _time_branch(x)                      # exact G_time @ x, [N, IN]
    wbv = np.ascontiguousarray((ALPHA * weight).astype(bf))
    wtbv = np.ascontiguousarray(((1.0 - ALPHA) * weight_time).astype(bf))
    _mark("host prep")

    nc = _build_nc(c_shift)
    _mark("build+schedule")
    in_maps = []
    for c in range(NCORES):
        sl = slice(c * NLOC, (c + 1) * NLOC)
        in_maps.append(dict(
            qT=np.ascontiguousarray(qTv[:, sl]),
            kT=kTv, xb=xbf,
            trt=np.ascontiguousarray(trows[sl].T.astype(bf)),
            wb=wbv, wtb=wtbv,
        ))

    _mark("in_maps")
    res = run_bass_kernel_spmd(nc, in_maps, list(range(NCORES)))
    _mark("device run (compile+ship+exec)")
    out = np.empty((N, NOUT), np.float32)
    for c in range(NCORES):
        out[c * NLOC:(c + 1) * NLOC] = res.results[c]["outT"].T
    _mark("epilogue")
    return out


def kernel(**inputs):
    if _IMPORTS_OK:
        try:
            out = _device_kernel(**inputs)
            if not np.isfinite(out).all():
                raise FloatingPointError("non-finite values in device output")
            return out.astype(np.asarray(inputs["x"]).dtype)
        except Exception:
            traceback.print_exc()
            sys.stderr.write("device path failed; using host fallback\n")
    return _host_fallback(**inputs)


# revision 18
# speedup vs baseline: 7.3947x; 1.3531x over previous
"""Trainium2 Bass kernel for nn_Graph_Layer_44787918963014 (gnn_message_passing).

out = ALPHA * softmax(q k^T) @ x @ weight + (1-ALPHA) * G_time @ x @ weight_time
with q = x@W0.T, k = x@W1.T, G_time the normalized (n-|i-j|) Toeplitz affinity.

Strategy (8 NeuronCores, row-sharded: core c owns queries [c*1024, (c+1)*1024)):
  host prep : q/k projections (cheap [N,512]@[512,128] GEMMs, shipped fp32);
              global constant score shift c (softmax-invariant, estimated from
              sampled rows; fp32 exp has ~87 of headroom each side);
              G_time @ x computed EXACTLY in O(N*D) via prefix sums (Toeplitz
              structure), so the time branch needs no N x N work.
  device    : per j-block of 128 keys -> scores S^T[j,m] via one fp32 matmul
              into PSUM; exp(S^T - c) on ACT -> bf16 E^T; Z partials (DVE);
              U^T[d,m] += x_j^T E_j accumulated across all 64 j-blocks directly
              in PSUM (no SBUF flushes); Z partition-broadcast via all-ones
              matmul; 1/Z on DVE; single fused projection
              outT = [a*W; (1-a)*Wt]^T @ [U^T/Z; trT].
  host epi  : out[rows] = outT.T  (transpose only).

Self-contained: shapes hardcoded, no sibling imports. Falls back to an exact
blocked host computation if the device path fails for any reason.
"""
import os, sys, time, hashlib, traceback
import numpy as np

N, IN, FEAT, NOUT = 8192, 512, 128, 512
ALPHA = 0.5
NCORES = 8
NLOC = N // NCORES     # 1024 queries per core
P = 128
NBLK = N // P          # 64 key blocks
NH = NLOC // 512       # 2 query halves of 512 (PSUM bank width)
ND = IN // P           # 4 d-chunks of x features

_NEFF_CACHE_DIR = "/tmp/.bass_neff_cache"

try:
    import ml_dtypes
    from contextlib import ExitStack
    from concourse import bass, tile, mybir, bass2jax
    from concourse.bass_utils import run_bass_kernel_spmd
    from concourse.vector_clock import ScopedClock, VectorClock
    _IMPORTS_OK = True
except Exception:
    traceback.print_exc()
    _IMPORTS_OK = False


def _host_fallback(x, W0, W1, weight, weight_time):
    x = np.asarray(x, np.float32)
    q = x @ np.asarray(W0, np.float32).T
    k = np.asarray(np.asarray(W1, np.float32) @ x.T)        # [FEAT, N]
    out = np.empty((N, NOUT), np.float32)
    w = np.asarray(weight, np.float32)
    blk = 1024
    for i0 in range(0, N, blk):
        s = q[i0:i0 + blk] @ k                               # [blk, N]
        s -= s.max(1, keepdims=True)
        np.exp(s, out=s)
        s /= s.sum(1, keepdims=True)
        out[i0:i0 + blk] = ALPHA * ((s @ x) @ w)
    out += _time_branch(x) @ ((1.0 - ALPHA) * np.asarray(weight_time, np.float32))
    return out


def _time_branch(x):
    """G_time @ x computed exactly via prefix sums (Toeplitz structure).
    fp32 cumsums: partial sums stay O(300), so the error reaching T after the
    /S_i (~5e7) normalization is ~1e-7 -- far below the bf16 shipping dtype."""
    xf = np.asarray(x, np.float32)
    i = np.arange(N, dtype=np.float32)
    i64 = np.arange(N, dtype=np.float64)
    Pc = np.cumsum(xf, axis=0)                   # P_i = sum_{j<=i} x_j
    Qc = np.cumsum(i[:, None] * xf, axis=0)      # Q_i = sum_{j<=i} j*x_j
    Pn = Pc[-1].astype(np.float64)
    Qn = Qc[-1].astype(np.float64)
    A = 2.0 * (i[:, None] * Pc - Qc) + (Qn[None, :] - i64[:, None] * Pn[None, :])
    S = N * N - (i64 * (i64 + 1) / 2 + (N - 1 - i64) * (N - i64) / 2)
    T = (N * Pn[None, :] - A) / S[:, None]
    return T.astype(np.float32)


def _install_compile_shims():
    """Two shims around the BIR -> NEFF compile:
    1. Wait splitter: this walrus build rejects instructions carrying more
       than ~2 sync waits. Any instruction with >1 on_wait keeps its last
       wait; the rest become standalone EventSemaphore instructions
       immediately before it on the same engine (identical semantics:
       engine program order runs them first).
    2. NEFF disk cache keyed on the (rewritten) BIR bytes, so repeat
       invocations in fresh processes skip walrus entirely.
    """
    if getattr(bass2jax, "_compile_shims_installed", False):
        return
    import orjson

    def _split(bir):
        ctr = [0]

        def fix_block(b):
            out = []
            for inst in b.get("instructions", []):
                si = inst.get("sync_info")
                waits = (si or {}).get("on_wait") or []
                if len(waits) > 1 and inst.get("engine") not in (None, "Unassigned"):
                    extra, keep = waits[:-1], waits[-1:]
                    for w in extra:
                        ctr[0] += 1
                        out.append({
                            "debug": inst.get("debug"),
                            "engine": inst["engine"],
                            "ins": [], "outs": [],
                            "name": f"wsplit-{ctr[0]}",
                            "opcode": "EventSemaphore",
                            "sync_info": {"on_update": [], "on_wait": [w]},
                        })
                    si["on_wait"] = keep
                out.append(inst)
            b["instructions"] = out
            for sb in b.get("blocks", []):
                fix_block(sb)

        for fn in bir["functions"]:
            for b in fn["blocks"]:
                fix_block(b)
        return ctr[0]

    orig = bass2jax.compile_bir_kernel

    def wrapped(bir_json, tmpdir, neff_name="file.neff"):
        try:
            bir = orjson.loads(bir_json)
            if _split(bir):
                bir_json = orjson.dumps(bir)
        except Exception:
            traceback.print_exc()
        cache_path = None
        try:
            os.makedirs(_NEFF_CACHE_DIR, exist_ok=True)
            key = hashlib.sha256(bir_json).hexdigest()
            cache_path = os.path.join(_NEFF_CACHE_DIR, f"{key}.neff")
            if os.path.exists(cache_path):
                dst = os.path.join(tmpdir, neff_name)
                with open(cache_path, "rb") as f, open(dst, "wb") as g:
                    g.write(f.read())
                sys.stderr.write("[ktiming] neff cache hit\n")
                return dst
        except Exception:
            traceback.print_exc()
        t = time.time()
        neff_path = orig(bir_json, tmpdir, neff_name)
        sys.stderr.write(f"[ktiming] walrus compile: {time.time()-t:.2f}s\n")
        try:
            if cache_path:
                tmp = cache_path + ".tmp"
                with open(neff_path, "rb") as f, open(tmp, "wb") as g:
                    g.write(f.read())
                os.replace(tmp, cache_path)
        except Exception:
            traceback.print_exc()
        return neff_path

    bass2jax.compile_bir_kernel = wrapped
    bass2jax._compile_shims_installed = True


class _PatchedTC(tile.TileContext if _IMPORTS_OK else object):
    """Tail drain emits one drain per proc (>2 sync waits on one CTRL inst
    is rejected by this walrus build)."""

    def _drain_and_barrier(self, tick_clock, wait_clock):
        gc = tick_clock.global_clock
        n = len(gc)
        for p in range(n):
            t = gc[p]
            if t <= 0:
                continue
            vec = [0] * n
            vec[p] = t
            d = self.nc.sync.drain()
            wait_clock.add_sem_waits(d.ins, ScopedClock({None: VectorClock(vec)}))
        self.nc.all_engine_barrier()
        popped = self.nc._tile_sem_poison_stack.pop()
        assert popped is self._sem_poison
        self.nc.clear_and_free_semaphores(list(self.sems.allocated().values()))
        self.nc.all_engine_barrier()


def _build_nc(c_shift, use_collectives):
    F32 = mybir.dt.float32
    BF16 = mybir.dt.bfloat16
    Exp = mybir.ActivationFunctionType.Exp
    ADD = mybir.AluOpType.add
    MUL = mybir.AluOpType.mult

    nc = bass.Bass("TRN2", num_devices=NCORES)
    qT = nc.declare_dram_parameter("qT", [FEAT, NLOC], F32, isOutput=False)
    if use_collectives:
        # sharded inputs; full x / kT assembled on device over NeuronLink
        kts = nc.declare_dram_parameter("kts", [FEAT, NLOC], F32, isOutput=False)
        xbs = nc.declare_dram_parameter("xbs", [NLOC, IN], BF16, isOutput=False)
    else:
        kT = nc.declare_dram_parameter("kT", [FEAT, N], F32, isOutput=False)
        xb = nc.declare_dram_parameter("xb", [N, IN], BF16, isOutput=False)
    trt = nc.declare_dram_parameter("trt", [IN, NLOC], BF16, isOutput=False)
    wb = nc.declare_dram_parameter("wb", [IN, NOUT], BF16, isOutput=False)
    wtb = nc.declare_dram_parameter("wtb", [IN, NOUT], BF16, isOutput=False)
    outT = nc.declare_dram_parameter("outT", [NOUT, NLOC], F32, isOutput=True)

    with _PatchedTC(nc) as tc, ExitStack() as ctx:
        cst = ctx.enter_context(tc.tile_pool(name="cst", bufs=1))
        xpool = ctx.enter_context(tc.tile_pool(name="xp", bufs=1))
        epool = ctx.enter_context(tc.tile_pool(name="ep", bufs=4))
        upool = ctx.enter_context(tc.tile_pool(name="up", bufs=1, space="PSUM"))
        spool = ctx.enter_context(tc.tile_pool(name="sp", bufs=2, space="PSUM"))
        ppool = ctx.enter_context(tc.tile_pool(name="pp", bufs=2, space="PSUM"))
        usbp = ctx.enter_context(tc.tile_pool(name="usb", bufs=2))
        misc = ctx.enter_context(tc.tile_pool(name="misc", bufs=1))

        qt = cst.tile([FEAT, NLOC], F32, name="qt")
        nc.sync.dma_start(qt[:], qT[:])
        kt = cst.tile([FEAT, N], F32, name="kt")
        xt = []

        if use_collectives:
            dram = ctx.enter_context(tc.tile_pool(name="dram", bufs=1, space="DRAM"))
            kin = dram.tile([FEAT, NLOC], F32, name="kin", tag="kin")
            kg = dram.tile([NCORES * FEAT, NLOC], F32, name="kg", tag="kg")
            xin = dram.tile([NLOC, IN], BF16, name="xin", tag="xin")
            xg = dram.tile([N, IN], BF16, name="xg", tag="xg")
            nc.gpsimd.dma_start(kin[:], kts[:])
            nc.gpsimd.dma_start(xin[:], xbs[:])
            groups = [list(range(NCORES))]
            nc.gpsimd.collective_compute(
                "AllGather", mybir.AluOpType.bypass, replica_groups=groups,
                ins=[kin.opt()], outs=[kg.opt()])
            nc.gpsimd.collective_compute(
                "AllGather", mybir.AluOpType.bypass, replica_groups=groups,
                ins=[xin.opt()], outs=[xg.opt()])
            for c in range(NCORES):
                nc.sync.dma_start(kt[:, c * NLOC:(c + 1) * NLOC],
                                  kg[c * FEAT:(c + 1) * FEAT, :])
            for b in range(NBLK):
                t = xpool.tile([P, IN], BF16, name=f"x{b}", tag=f"x{b}")
                nc.sync.dma_start(t[:], xg[b * P:(b + 1) * P, :])
                xt.append(t)
        else:
            nc.sync.dma_start(kt[:], kT[:])
            for b in range(NBLK):
                t = xpool.tile([P, IN], BF16, name=f"x{b}", tag=f"x{b}")
                nc.sync.dma_start(t[:], xb[b * P:(b + 1) * P, :])
                xt.append(t)
        trtt = []
        for dd in range(ND):
            t = cst.tile([P, NLOC], BF16, name=f"tr{dd}", tag=f"tr{dd}")
            nc.sync.dma_start(t[:], trt[dd * P:(dd + 1) * P, :])
            trtt.append(t)
        wbt, wtbt = [], []
        for dd in range(ND):
            t = cst.tile([P, NOUT], BF16, name=f"wb{dd}", tag=f"wb{dd}")
            nc.sync.dma_start(t[:], wb[dd * P:(dd + 1) * P, :])
            wbt.append(t)
            t2 = cst.tile([P, NOUT], BF16, name=f"wt{dd}", tag=f"wt{dd}")
            nc.sync.dma_start(t2[:], wtb[dd * P:(dd + 1) * P, :])
            wtbt.append(t2)

        ones128 = misc.tile([P, P], F32, name="ones128")
        nc.vector.memset(ones128[:], 1.0)
        bconst = misc.tile([P, 1], F32, name="bconst")
        nc.vector.memset(bconst[:], -float(c_shift))
        zacc = misc.tile([P, NLOC], F32, name="zacc")
        nc.vector.memset(zacc[:], 0.0)
        outsb = [misc.tile([P, NLOC], F32, name=f"ou{oo}", tag=f"ou{oo}")
                 for oo in range(ND)]

        for h in range(NH):
            msl = slice(h * 512, h * 512 + 512)
            ups = [upool.tile([P, 512], F32, name=f"u{h}_{dd}", tag=f"u{dd}")
                   for dd in range(ND)]
            for b in range(NBLK):
                jsl = slice(b * P, (b + 1) * P)
                sp = spool.tile([P, 512], F32, name=f"s{h}_{b}", tag="s")
                nc.tensor.matmul(sp[:], kt[:, jsl], qt[:, msl], start=True, stop=True)
                et = epool.tile([P, 512], BF16, name=f"e{h}_{b}", tag="e")
                nc.scalar.activation(et[:], sp[:], Exp, bias=bconst[:])
                nc.vector.tensor_tensor(zacc[:, msl], zacc[:, msl], et[:], ADD)
                for dd in range(ND):
                    dsl = slice(dd * P, (dd + 1) * P)
                    nc.tensor.matmul(ups[dd][:], xt[b][:, dsl], et[:],
                                     start=(b == 0), stop=(b == NBLK - 1))
            # Z broadcast to all partitions in one matmul: (ones 128x128) @ zacc
            zps = ppool.tile([P, 512], F32, name=f"zp{h}", tag="proj")
            nc.tensor.matmul(zps[:], ones128[:], zacc[:, msl], start=True, stop=True)
            zrb = usbp.tile([P, 512], F32, name=f"zr{h}", tag="zr")
            nc.vector.reciprocal(zrb[:], zps[:])
            # scale U^T by 1/Z (frees the U PSUM banks), cast to bf16
            usb = []
            for dd in range(ND):
                t = usbp.tile([P, 512], BF16, name=f"us{h}_{dd}", tag=f"us{dd}")
                nc.vector.tensor_tensor(t[:], ups[dd][:], zrb[:], MUL)
                usb.append(t)
            # fused projection: outT[o, m] = sum_d [wb;wtb][d,o] * [U/Z; trT][d,m]
            for oo in range(ND):
                osl = slice(oo * P, (oo + 1) * P)
                po = ppool.tile([P, 512], F32, name=f"po{h}_{oo}", tag="proj")
                for dd in range(ND):
                    nc.tensor.matmul(po[:], wbt[dd][:, osl], usb[dd][:],
                                     start=(dd == 0), stop=False)
                for dd in range(ND):
                    nc.tensor.matmul(po[:], wtbt[dd][:, osl], trtt[dd][:, msl],
                                     start=False, stop=(dd == ND - 1))
                nc.scalar.activation(outsb[oo][:, msl], po[:],
                                     mybir.ActivationFunctionType.Copy)
        for oo in range(ND):
            nc.sync.dma_start(outT[oo * P:(oo + 1) * P, :], outsb[oo][:])
    return nc


def _device_kernel(x, W0, W1, weight, weight_time, use_collectives=True):
    _install_compile_shims()
    _t0 = time.time()

    def _mark(m):
        sys.stderr.write(f"[ktiming] {m}: {time.time()-_t0:.2f}s\n")
        sys.stderr.flush()

    bf = ml_dtypes.bfloat16
    x = np.asarray(x, np.float32)
    W0 = np.asarray(W0, np.float32)
    W1 = np.asarray(W1, np.float32)
    weight = np.asarray(weight, np.float32)
    weight_time = np.asarray(weight_time, np.float32)

    qTv = np.ascontiguousarray((x @ W0.T).T)     # [FEAT, N] fp32
    kTv = np.ascontiguousarray(W1 @ x.T)         # [FEAT, N] fp32
    xbf = x.astype(bf)

    # constant softmax shift: sampled row maxima + margin (fp32 exp has ~87 of
    # headroom on either side, so the sampling error margin is enormous)
    samp = qTv[:, ::512].T @ kTv                 # [16, N] scores
    c_shift = float(samp.max()) + 8.0

    trows = _time_branch(x)                      # exact G_time @ x, [N, IN]
    wbv = np.ascontiguousarray((ALPHA * weight).astype(bf))
    wtbv = np.ascontiguousarray(((1.0 - ALPHA) * weight_time).astype(bf))
    _mark("host prep")

    nc = _build_nc(c_shift, use_collectives)
    _mark("build+schedule")
    in_maps = []
    for c in range(NCORES):
        sl = slice(c * NLOC, (c + 1) * NLOC)
        m = dict(
            qT=np.ascontiguousarray(qTv[:, sl]),
            trt=np.ascontiguousarray(trows[sl].T.astype(bf)),
            wb=wbv, wtb=wtbv,
        )
        if use_collectives:
            m["kts"] = np.ascontiguousarray(kTv[:, sl])
            m["xbs"] = np.ascontiguousarray(xbf[sl])
        else:
            m["kT"] = kTv
            m["xb"] = xbf
        in_maps.append(m)

    _mark("in_maps")
    res = run_bass_kernel_spmd(nc, in_maps, list(range(NCORES)))
    _mark("device run (compile+ship+exec)")
    out = np.empty((N, NOUT), np.float32)
    for c in range(NCORES):
        out[c * NLOC:(c + 1) * NLOC] = res.results[c]["outT"].T
    _mark("epilogue")
    return out


def kernel(**inputs):
    if _IMPORTS_OK:
        for use_cc in (True, False):
            try:
                out = _device_kernel(**inputs, use_collectives=use_cc)
                if not np.isfinite(out).all():
                    raise FloatingPointError("non-finite values in device output")
                return out.astype(np.asarray(inputs["x"]).dtype)
            except Exception:
                traceback.print_exc()
                sys.stderr.write(
                    f"device path (collectives={use_cc}) failed; trying next\n")
    return _host_fallback(**inputs)


# revision 24
# speedup vs baseline: 10.1268x; 1.3695x over previous
"""Trainium2 Bass kernel for nn_Graph_Layer_44787918963014 (gnn_message_passing).

out = ALPHA * softmax(q k^T) @ x @ weight + (1-ALPHA) * G_time @ x @ weight_time
with q = x@W0.T, k = x@W1.T, G_time the normalized (n-|i-j|) Toeplitz affinity.

Strategy (8 NeuronCores, row-sharded: core c owns queries [c*1024, (c+1)*1024)):
  host prep : q/k projections (cheap [N,512]@[512,128] GEMMs, shipped fp32);
              global constant score shift c (softmax-invariant, estimated from
              sampled rows; fp32 exp has ~87 of headroom each side);
              G_time @ x computed EXACTLY in O(N*D) via prefix sums (Toeplitz
              structure), so the time branch needs no N x N work.
  device    : per j-block of 128 keys -> scores S^T[j,m] via one fp32 matmul
              into PSUM; exp(S^T - c) on ACT -> bf16 E^T; Z partials (DVE);
              U^T[d,m] += x_j^T E_j accumulated across all 64 j-blocks directly
              in PSUM (no SBUF flushes); Z partition-broadcast via all-ones
              matmul; 1/Z on DVE; single fused projection
              outT = [a*W; (1-a)*Wt]^T @ [U^T/Z; trT].
  host epi  : out[rows] = outT.T  (transpose only).

Self-contained: shapes hardcoded, no sibling imports. Falls back to an exact
blocked host computation if the device path fails for any reason.
"""
import os, sys, time, hashlib, traceback
import numpy as np

N, IN, FEAT, NOUT = 8192, 512, 128, 512
ALPHA = 0.5
NCORES = 8
NLOC = N // NCORES     # 1024 queries per core
P = 128
NBLK = N // P          # 64 key blocks
NH = NLOC // 512       # 2 query halves of 512 (PSUM bank width)
ND = IN // P           # 4 d-chunks of x features

_NEFF_CACHE_DIR = "/tmp/.bass_neff_cache"

try:
    import ml_dtypes
    from contextlib import ExitStack
    from concourse import bass, tile, mybir, bass2jax
    from concourse.bass_utils import run_bass_kernel_spmd
    from concourse.vector_clock import ScopedClock, VectorClock
    _IMPORTS_OK = True
except Exception:
    traceback.print_exc()
    _IMPORTS_OK = False


def _host_fallback(x, W0, W1, weight, weight_time):
    x = np.asarray(x, np.float32)
    q = x @ np.asarray(W0, np.float32).T
    k = np.asarray(np.asarray(W1, np.float32) @ x.T)        # [FEAT, N]
    out = np.empty((N, NOUT), np.float32)
    w = np.asarray(weight, np.float32)
    blk = 1024
    for i0 in range(0, N, blk):
        s = q[i0:i0 + blk] @ k                               # [blk, N]
        s -= s.max(1, keepdims=True)
        np.exp(s, out=s)
        s /= s.sum(1, keepdims=True)
        out[i0:i0 + blk] = ALPHA * ((s @ x) @ w)
    out += _time_branch(x) @ ((1.0 - ALPHA) * np.asarray(weight_time, np.float32))
    return out


def _time_branch(x):
    """G_time @ x computed exactly via prefix sums (Toeplitz structure).
    fp32 cumsums: partial sums stay O(300), so the error reaching T after the
    /S_i (~5e7) normalization is ~1e-7 -- far below the bf16 shipping dtype."""
    xf = np.asarray(x, np.float32)
    i = np.arange(N, dtype=np.float32)
    i64 = np.arange(N, dtype=np.float64)
    Pc = np.cumsum(xf, axis=0)                   # P_i = sum_{j<=i} x_j
    Qc = np.cumsum(i[:, None] * xf, axis=0)      # Q_i = sum_{j<=i} j*x_j
    Pn = Pc[-1].astype(np.float64)
    Qn = Qc[-1].astype(np.float64)
    A = 2.0 * (i[:, None] * Pc - Qc) + (Qn[None, :] - i64[:, None] * Pn[None, :])
    S = N * N - (i64 * (i64 + 1) / 2 + (N - 1 - i64) * (N - i64) / 2)
    T = (N * Pn[None, :] - A) / S[:, None]
    return T.astype(np.float32)


def _install_compile_shims():
    """Two shims around the BIR -> NEFF compile:
    1. Wait splitter: this walrus build rejects instructions carrying more
       than ~2 sync waits. Any instruction with >1 on_wait keeps its last
       wait; the rest become standalone EventSemaphore instructions
       immediately before it on the same engine (identical semantics:
       engine program order runs them first).
    2. NEFF disk cache keyed on the (rewritten) BIR bytes, so repeat
       invocations in fresh processes skip walrus entirely.
    """
    if getattr(bass2jax, "_compile_shims_installed", False):
        return
    import orjson

    def _split(bir):
        ctr = [0]

        def fix_block(b):
            out = []
            for inst in b.get("instructions", []):
                si = inst.get("sync_info")
                waits = (si or {}).get("on_wait") or []
                if len(waits) > 1 and inst.get("engine") not in (None, "Unassigned"):
                    extra, keep = waits[:-1], waits[-1:]
                    for w in extra:
                        ctr[0] += 1
                        out.append({
                            "debug": inst.get("debug"),
                            "engine": inst["engine"],
                            "ins": [], "outs": [],
                            "name": f"wsplit-{ctr[0]}",
                            "opcode": "EventSemaphore",
                            "sync_info": {"on_update": [], "on_wait": [w]},
                        })
                    si["on_wait"] = keep
                out.append(inst)
            b["instructions"] = out
            for sb in b.get("blocks", []):
                fix_block(sb)

        for fn in bir["functions"]:
            for b in fn["blocks"]:
                fix_block(b)
        return ctr[0]

    orig = bass2jax.compile_bir_kernel

    def wrapped(bir_json, tmpdir, neff_name="file.neff"):
        try:
            bir = orjson.loads(bir_json)
            if _split(bir):
                bir_json = orjson.dumps(bir)
        except Exception:
            traceback.print_exc()
        cache_path = None
        try:
            os.makedirs(_NEFF_CACHE_DIR, exist_ok=True)
            key = hashlib.sha256(bir_json).hexdigest()
            cache_path = os.path.join(_NEFF_CACHE_DIR, f"{key}.neff")
            if os.path.exists(cache_path):
                dst = os.path.join(tmpdir, neff_name)
                with open(cache_path, "rb") as f, open(dst, "wb") as g:
                    g.write(f.read())
                sys.stderr.write("[ktiming] neff cache hit\n")
                return dst
        except Exception:
            traceback.print_exc()
        t = time.time()
        neff_path = orig(bir_json, tmpdir, neff_name)
        sys.stderr.write(f"[ktiming] walrus compile: {time.time()-t:.2f}s\n")
        try:
            if cache_path:
                tmp = cache_path + ".tmp"
                with open(neff_path, "rb") as f, open(tmp, "wb") as g:
                    g.write(f.read())
                os.replace(tmp, cache_path)
        except Exception:
            traceback.print_exc()
        return neff_path

    bass2jax.compile_bir_kernel = wrapped
    bass2jax._compile_shims_installed = True


class _PatchedTC(tile.TileContext if _IMPORTS_OK else object):
    """Tail drain emits one drain per proc (>2 sync waits on one CTRL inst
    is rejected by this walrus build)."""

    def _drain_and_barrier(self, tick_clock, wait_clock):
        gc = tick_clock.global_clock
        n = len(gc)
        for p in range(n):
            t = gc[p]
            if t <= 0:
                continue
            vec = [0] * n
            vec[p] = t
            d = self.nc.sync.drain()
            wait_clock.add_sem_waits(d.ins, ScopedClock({None: VectorClock(vec)}))
        self.nc.all_engine_barrier()
        popped = self.nc._tile_sem_poison_stack.pop()
        assert popped is self._sem_poison
        self.nc.clear_and_free_semaphores(list(self.sems.allocated().values()))
        self.nc.all_engine_barrier()


def _build_nc(c_shift, use_collectives):
    F32 = mybir.dt.float32
    BF16 = mybir.dt.bfloat16
    Exp = mybir.ActivationFunctionType.Exp
    ADD = mybir.AluOpType.add
    MUL = mybir.AluOpType.mult

    nc = bass.Bass("TRN2", num_devices=NCORES)
    qT = nc.declare_dram_parameter("qT", [FEAT, NLOC], F32, isOutput=False)
    if use_collectives:
        # sharded inputs; full x / kT / weights assembled on device (NeuronLink)
        kts = nc.declare_dram_parameter("kts", [FEAT, NLOC], F32, isOutput=False)
        xbs = nc.declare_dram_parameter("xbs", [NLOC, IN], BF16, isOutput=False)
        wbs = nc.declare_dram_parameter("wbs", [IN // NCORES, NOUT], BF16, isOutput=False)
        wtbs = nc.declare_dram_parameter("wtbs", [IN // NCORES, NOUT], BF16, isOutput=False)
    else:
        kT = nc.declare_dram_parameter("kT", [FEAT, N], F32, isOutput=False)
        xb = nc.declare_dram_parameter("xb", [N, IN], BF16, isOutput=False)
        wb = nc.declare_dram_parameter("wb", [IN, NOUT], BF16, isOutput=False)
        wtb = nc.declare_dram_parameter("wtb", [IN, NOUT], BF16, isOutput=False)
    trt = nc.declare_dram_parameter("trt", [IN, NLOC], BF16, isOutput=False)
    outT = nc.declare_dram_parameter("outT", [NOUT, NLOC], BF16, isOutput=True)

    with _PatchedTC(nc) as tc, ExitStack() as ctx:
        cst = ctx.enter_context(tc.tile_pool(name="cst", bufs=1))
        xpool = ctx.enter_context(tc.tile_pool(name="xp", bufs=1))
        epool = ctx.enter_context(tc.tile_pool(name="ep", bufs=4))
        upool = ctx.enter_context(tc.tile_pool(name="up", bufs=1, space="PSUM"))
        spool = ctx.enter_context(tc.tile_pool(name="sp", bufs=2, space="PSUM"))
        ppool = ctx.enter_context(tc.tile_pool(name="pp", bufs=2, space="PSUM"))
        usbp = ctx.enter_context(tc.tile_pool(name="usb", bufs=2))
        misc = ctx.enter_context(tc.tile_pool(name="misc", bufs=1))

        qt = cst.tile([FEAT, NLOC], F32, name="qt")
        nc.sync.dma_start(qt[:], qT[:])
        kt = cst.tile([FEAT, N], F32, name="kt")
        xt = []

        if use_collectives:
            dram = ctx.enter_context(tc.tile_pool(name="dram", bufs=1, space="DRAM"))
            kin = dram.tile([FEAT, NLOC], F32, name="kin", tag="kin")
            kg = dram.tile([NCORES * FEAT, NLOC], F32, name="kg", tag="kg")
            xin = dram.tile([NLOC, IN], BF16, name="xin", tag="xin")
            xg = dram.tile([N, IN], BF16, name="xg", tag="xg")
            win = dram.tile([IN // NCORES, NOUT], BF16, name="win", tag="win")
            wg = dram.tile([IN, NOUT], BF16, name="wg", tag="wg")
            wtin = dram.tile([IN // NCORES, NOUT], BF16, name="wtin", tag="wtin")
            wtg = dram.tile([IN, NOUT], BF16, name="wtg", tag="wtg")
            nc.gpsimd.dma_start(kin[:], kts[:])
            nc.gpsimd.dma_start(xin[:], xbs[:])
            nc.gpsimd.dma_start(win[:], wbs[:])
            nc.gpsimd.dma_start(wtin[:], wtbs[:])
            groups = [list(range(NCORES))]
            for src, dst in ((kin, kg), (xin, xg), (win, wg), (wtin, wtg)):
                nc.gpsimd.collective_compute(
                    "AllGather", mybir.AluOpType.bypass, replica_groups=groups,
                    ins=[src.opt()], outs=[dst.opt()])
            for c in range(NCORES):
                nc.sync.dma_start(kt[:, c * NLOC:(c + 1) * NLOC],
                                  kg[c * FEAT:(c + 1) * FEAT, :])
            for b in range(NBLK):
                t = xpool.tile([P, IN], BF16, name=f"x{b}", tag=f"x{b}")
                nc.sync.dma_start(t[:], xg[b * P:(b + 1) * P, :])
                xt.append(t)
            wb, wtb = wg, wtg
        else:
            nc.sync.dma_start(kt[:], kT[:])
            for b in range(NBLK):
                t = xpool.tile([P, IN], BF16, name=f"x{b}", tag=f"x{b}")
                nc.sync.dma_start(t[:], xb[b * P:(b + 1) * P, :])
                xt.append(t)
        trtt = []
        for dd in range(ND):
            t = cst.tile([P, NLOC], BF16, name=f"tr{dd}", tag=f"tr{dd}")
            nc.sync.dma_start(t[:], trt[dd * P:(dd + 1) * P, :])
            trtt.append(t)
        wbt, wtbt = [], []
        for dd in range(ND):
            t = cst.tile([P, NOUT], BF16, name=f"wb{dd}", tag=f"wb{dd}")
            nc.sync.dma_start(t[:], wb[dd * P:(dd + 1) * P, :])
            wbt.append(t)
            t2 = cst.tile([P, NOUT], BF16, name=f"wt{dd}", tag=f"wt{dd}")
            nc.sync.dma_start(t2[:], wtb[dd * P:(dd + 1) * P, :])
            wtbt.append(t2)

        ones128 = misc.tile([P, P], F32, name="ones128")
        nc.vector.memset(ones128[:], 1.0)
        bconst = misc.tile([P, 1], F32, name="bconst")
        nc.vector.memset(bconst[:], -float(c_shift))
        zacc = misc.tile([P, NLOC], F32, name="zacc")
        nc.vector.memset(zacc[:], 0.0)
        outsb = [misc.tile([P, NLOC], BF16, name=f"ou{oo}", tag=f"ou{oo}")
                 for oo in range(ND)]

        for h in range(NH):
            msl = slice(h * 512, h * 512 + 512)
            ups = [upool.tile([P, 512], F32, name=f"u{h}_{dd}", tag=f"u{dd}")
                   for dd in range(ND)]
            for b in range(NBLK):
                jsl = slice(b * P, (b + 1) * P)
                sp = spool.tile([P, 512], F32, name=f"s{h}_{b}", tag="s")
                nc.tensor.matmul(sp[:], kt[:, jsl], qt[:, msl], start=True, stop=True)
                et = epool.tile([P, 512], BF16, name=f"e{h}_{b}", tag="e")
                nc.scalar.activation(et[:], sp[:], Exp, bias=bconst[:])
                nc.vector.tensor_tensor(zacc[:, msl], zacc[:, msl], et[:], ADD)
                for dd in range(ND):
                    dsl = slice(dd * P, (dd + 1) * P)
                    nc.tensor.matmul(ups[dd][:], xt[b][:, dsl], et[:],
                                     start=(b == 0), stop=(b == NBLK - 1))
            # Z broadcast to all partitions in one matmul: (ones 128x128) @ zacc
            zps = ppool.tile([P, 512], F32, name=f"zp{h}", tag="proj")
            nc.tensor.matmul(zps[:], ones128[:], zacc[:, msl], start=True, stop=True)
            zrb = usbp.tile([P, 512], F32, name=f"zr{h}", tag="zr")
            nc.vector.reciprocal(zrb[:], zps[:])
            # scale U^T by 1/Z (frees the U PSUM banks), cast to bf16
            usb = []
            for dd in range(ND):
                t = usbp.tile([P, 512], BF16, name=f"us{h}_{dd}", tag=f"us{dd}")
                nc.vector.tensor_tensor(t[:], ups[dd][:], zrb[:], MUL)
                usb.append(t)
            # fused projection: outT[o, m] = sum_d [wb;wtb][d,o] * [U/Z; trT][d,m]
            for oo in range(ND):
                osl = slice(oo * P, (oo + 1) * P)
                po = ppool.tile([P, 512], F32, name=f"po{h}_{oo}", tag="proj")
                for dd in range(ND):
                    nc.tensor.matmul(po[:], wbt[dd][:, osl], usb[dd][:],
                                     start=(dd == 0), stop=False)
                for dd in range(ND):
                    nc.tensor.matmul(po[:], wtbt[dd][:, osl], trtt[dd][:, msl],
                                     start=False, stop=(dd == ND - 1))
                nc.scalar.activation(outsb[oo][:, msl], po[:],
                                     mybir.ActivationFunctionType.Copy)
        for oo in range(ND):
            nc.sync.dma_start(outT[oo * P:(oo + 1) * P, :], outsb[oo][:])
    return nc


def _device_kernel(x, W0, W1, weight, weight_time, use_collectives=True):
    _install_compile_shims()
    _t0 = time.time()

    def _mark(m):
        sys.stderr.write(f"[ktiming] {m}: {time.time()-_t0:.2f}s\n")
        sys.stderr.flush()

    bf = ml_dtypes.bfloat16
    x = np.asarray(x, np.float32)
    W0 = np.asarray(W0, np.float32)
    W1 = np.asarray(W1, np.float32)
    weight = np.asarray(weight, np.float32)
    weight_time = np.asarray(weight_time, np.float32)

    qTv = np.ascontiguousarray((x @ W0.T).T)     # [FEAT, N] fp32
    kTv = np.ascontiguousarray(W1 @ x.T)         # [FEAT, N] fp32
    xbf = x.astype(bf)

    # constant softmax shift: sampled row maxima + margin (fp32 exp has ~87 of
    # headroom on either side, so the sampling error margin is enormous)
    samp = qTv[:, ::512].T @ kTv                 # [16, N] scores
    c_shift = float(samp.max()) + 8.0

    trows = _time_branch(x)                      # exact G_time @ x, [N, IN]
    wbv = np.ascontiguousarray((ALPHA * weight).astype(bf))
    wtbv = np.ascontiguousarray(((1.0 - ALPHA) * weight_time).astype(bf))
    _mark("host prep")

    nc = _build_nc(c_shift, use_collectives)
    _mark("build+schedule")
    in_maps = []
    for c in range(NCORES):
        sl = slice(c * NLOC, (c + 1) * NLOC)
        m = dict(
            qT=np.ascontiguousarray(qTv[:, sl]),
            trt=np.ascontiguousarray(trows[sl].T.astype(bf)),
        )
        if use_collectives:
            m["kts"] = np.ascontiguousarray(kTv[:, sl])
            m["xbs"] = np.ascontiguousarray(xbf[sl])
            wsl = slice(c * (IN // NCORES), (c + 1) * (IN // NCORES))
            m["wbs"] = np.ascontiguousarray(wbv[wsl])
            m["wtbs"] = np.ascontiguousarray(wtbv[wsl])
        else:
            m["kT"] = kTv
            m["xb"] = xbf
            m["wb"] = wbv
            m["wtb"] = wtbv
        in_maps.append(m)

    _mark("in_maps")
    res = run_bass_kernel_spmd(nc, in_maps, list(range(NCORES)))
    _mark("device run (compile+ship+exec)")
    out = np.empty((N, NOUT), np.float32)
    for c in range(NCORES):
        out[c * NLOC:(c + 1) * NLOC] = res.results[c]["outT"].T.astype(np.float32)
    _mark("epilogue")
    return out


def kernel(**inputs):
    if _IMPORTS_OK:
        for use_cc in (True, False):
            try:
                out = _device_kernel(**inputs, use_collectives=use_cc)
                if not np.isfinite(out).all():
                    raise FloatingPointError("non-finite values in device output")
                return out.astype(np.asarray(inputs["x"]).dtype)
            except Exception:
                traceback.print_exc()
                sys.stderr.write(
                    f"device path (collectives={use_cc}) failed; trying next\n")
    return _host_fallback(**inputs)
